# revision 1
# baseline (speedup 1.0000x reference)
"""Trainium2 Bass kernel for the fern/sparse-table CTE model.

Strategy: data-parallel over batch N=32 across 8 cores (4 images each).
Per-site (image, fern, pixel) top-2 ambiguous-bit extraction is done with
masked segmented reduces on DVE; the T=4 table-row gather is served by one
dma_gather per (image, fern) from a host-prebuilt "pair table" whose 256B
elements hold the 4 rows (combinations of the two ambiguous bits) for a
packed (pair-id, base8) index - satisfying dma_gather's int16-index /
256B-element constraints with statically-sliced calls (no data-dependent
compaction). Votes are weighted and reduced on DVE; 2x2 average pooling and
the classifier run as PE matmuls.
"""
import os
import numpy as np
from contextlib import ExitStack

import concourse.bacc as bacc
import concourse.bass as bass
import concourse.tile as tile
from concourse import mybir
from concourse.bass_utils import run_bass_kernel_spmd

F32 = mybir.dt.float32
I16 = mybir.dt.int16
ALU = mybir.AluOpType
ACT = mybir.ActivationFunctionType

M, K, L = 8, 10, 6
D = 16                      # D_OUT
NCLS = 10
N, C, H, W = 32, 3, 64, 64
NCORES = 8
NI = N // NCORES            # images per core
HP = H + L - 1              # 69 padded
NPX = H * W                 # 4096
NT = NPX // 128             # 32 pixel tiles per image
NPAIR = 45
PTROWS = NPAIR * 256        # 11520 elements per fern


def _build_pair_table(table: np.ndarray) -> np.ndarray:
    """PT[m, pid*256+base8, 64] f32; rows j=ilo+2*ihi of the 256B element are
    table[m*1024 + unpack(base8;klo,khi) + ilo*2^klo + ihi*2^khi]."""
    tbl = table.reshape(M, 1024, D)
    PT = np.zeros((M, PTROWS, 4 * D), dtype=np.float32)
    base8 = np.arange(256)
    for khi in range(K):
        for klo in range(khi):
            pid = khi * (khi - 1) // 2 + klo
            rest = [k for k in range(K) if k not in (klo, khi)]
            unpacked = np.zeros(256, dtype=np.int64)
            for r, k in enumerate(rest):
                unpacked += ((base8 >> r) & 1) << k
            for ihi in range(2):
                for ilo in range(2):
                    j = ilo + 2 * ihi
                    rows = unpacked + ilo * (1 << klo) + ihi * (1 << khi)
                    PT[:, pid * 256 + base8, j * D:(j + 1) * D] = tbl[:, rows, :]
    return PT


def _host_consts(thresholds, w_pred, b_pred):
    pconst = np.zeros((128, 32), dtype=np.float32)
    pconst[:, 30] = -0.5
    pconst[:, 0:10] = (1 << np.arange(K)).astype(np.float32)[None, :]
    pconst[:, 10:20] = np.arange(K, dtype=np.float32)[None, :]
    pconst[:, 20:30] = np.arange(K, dtype=np.float32)[None, :] + 16.0
    thrneg = (-thresholds.reshape(M * K, 1)).astype(np.float32)
    ident = np.eye(128, dtype=np.float32)
    # pool lhsT: poolW[p, s*128 + s2*32 + w2] = 0.25 if s2==s and (p%64)//2==w2
    poolW = np.zeros((128, 4, 4, 32), dtype=np.float32)
    p = np.arange(128)
    for s in range(4):
        poolW[p, s, s, (p % 64) // 2] = 0.25
    poolW = poolW.reshape(128, 512)
    # classifier lhsT: wqT[p, c, cls] = w_pred[cls, d*1024 + (4g+s)*32 + w2]
    # with c = d*8+g, p = s*32+w2
    wq = w_pred.reshape(NCLS, D, 8, 4, 32)          # [cls, d, g, s, w2]
    wqT = np.transpose(wq, (3, 4, 1, 2, 0)).reshape(128, D * 8, NCLS)
    wqT = np.ascontiguousarray(wqT.reshape(128, D * 8 * NCLS)).astype(np.float32)
    bpred = b_pred.reshape(NCLS, 1).astype(np.float32)
    return pconst, thrneg, ident, poolW, wqT, bpred


def _build_kernel(c1, c2, dy1, dx1, dy2, dx2):
    """Build + compile the per-core kernel. Fern geometry is baked into the
    window-DMA access patterns at trace time."""
    nc = bacc.Bacc("TRN2", num_devices=NCORES, num_swdge_queues=4)

    xp_p = nc.declare_dram_parameter("xp", [NI, C, HP, HP], F32, isOutput=False)
    pt_p = nc.declare_dram_parameter("pt", [M, PTROWS, 4 * D], F32, isOutput=False)
    pc_p = nc.declare_dram_parameter("pconst", [128, 32], F32, isOutput=False)
    th_p = nc.declare_dram_parameter("thrneg", [M * K, 1], F32, isOutput=False)
    id_p = nc.declare_dram_parameter("ident", [128, 128], F32, isOutput=False)
    pw_p = nc.declare_dram_parameter("poolw", [128, 512], F32, isOutput=False)
    wq_p = nc.declare_dram_parameter("wqt", [128, D * 8 * NCLS], F32, isOutput=False)
    bp_p = nc.declare_dram_parameter("bpred", [NCLS, 1], F32, isOutput=False)
    out_p = nc.declare_dram_parameter("out", [NCLS, NI], F32, isOutput=True)

    with tile.TileContext(nc, num_cores=NCORES) as tc:
        with ExitStack() as ctx:
            cpool = ctx.enter_context(tc.tile_pool(name="consts", bufs=1))
            colp = ctx.enter_context(tc.tile_pool(name="col", bufs=1))
            bpxp = ctx.enter_context(tc.tile_pool(name="bpx", bufs=1))
            kp = ctx.enter_context(tc.tile_pool(name="kstage", bufs=1))
            mp = ctx.enter_context(tc.tile_pool(name="mstage", bufs=1))
            msc = ctx.enter_context(tc.tile_pool(name="mscratch", bufs=1))
            wp = ctx.enter_context(tc.tile_pool(name="wgidx", bufs=2))
            idxp = ctx.enter_context(tc.tile_pool(name="idx", bufs=2))
            vp = ctx.enter_context(tc.tile_pool(name="votes", bufs=2))
            wvp = ctx.enter_context(tc.tile_pool(name="wv", bufs=2))
            fp = ctx.enter_context(tc.tile_pool(name="feat", bufs=1))
            flp = ctx.enter_context(tc.tile_pool(name="flat", bufs=1))
            tps = ctx.enter_context(tc.tile_pool(name="tpsum", bufs=3, space="PSUM"))
            pps = ctx.enter_context(tc.tile_pool(name="ppsum", bufs=2, space="PSUM"))
            lps = ctx.enter_context(tc.tile_pool(name="lpsum", bufs=1, space="PSUM"))

            # ---- constants ----
            pconst = cpool.tile([128, 32], F32)
            nc.sync.dma_start(pconst[:], pc_p.ap())
            thr = cpool.tile([M * K, 1], F32)
            nc.sync.dma_start(thr[:], th_p.ap())
            ident = cpool.tile([128, 128], F32)
            nc.sync.dma_start(ident[:], id_p.ap())
            poolw = cpool.tile([128, 512], F32)
            nc.sync.dma_start(poolw[:], pw_p.ap())
            wqt = cpool.tile([128, D * 8 * NCLS], F32)
            nc.sync.dma_start(wqt[:], wq_p.ap())
            bpred = cpool.tile([NCLS, 1], F32)
            nc.sync.dma_start(bpred[:], bp_p.ap())

            def bc10(col):  # [128,10] const col -> [128,NT*M,10] broadcast
                v = pconst[:, col:col + 10]
                return v.unsqueeze(1).broadcast_to([128, NT * M, K])

            pow2b, iotab, iota16b = bc10(0), bc10(10), bc10(20)

            feat = fp.tile([128, NI, NT, D], F32)
            flatbuf = flp.tile([128, D, 8, NI], F32)

            for img in range(NI):
                # ---- stage A: windows -> soft bits (column layout) ----
                b1 = colp.tile([M * K, NPX], F32, tag="b1")
                b2 = colp.tile([M * K, NPX], F32, tag="b2")
                for m in range(M):
                    for k in range(K):
                        r = m * K + k
                        d1 = b1[r:r + 1, :].rearrange("p (a b) -> p a b", a=H, b=W)
                        s1_ = xp_p.ap()[img, int(c1[m, k]),
                                        int(dy1[m, k]):int(dy1[m, k]) + H,
                                        int(dx1[m, k]):int(dx1[m, k]) + W]
                        nc.sync.dma_start(d1, s1_.unsqueeze(0))
                        d2 = b2[r:r + 1, :].rearrange("p (a b) -> p a b", a=H, b=W)
                        s2_ = xp_p.ap()[img, int(c2[m, k]),
                                        int(dy2[m, k]):int(dy2[m, k]) + H,
                                        int(dx2[m, k]):int(dx2[m, k]) + W]
                        nc.scalar.dma_start(d2, s2_.unsqueeze(0))
                nc.vector.tensor_sub(b1[:], b1[:], b2[:])          # z
                nc.scalar.activation(b2[:], b1[:], ACT.Sigmoid, bias=thr[:], scale=1.0)

                # ---- transposes to pixel layout ----
                bpx = bpxp.tile([128, NT, M, K], F32, tag="bpx")
                done = 0
                while done < NT:
                    grp = min(6, NT - done)
                    tp = tps.tile([128, 480], F32, tag="tp")
                    for i in range(grp):
                        t_ = done + i
                        nc.tensor.transpose(
                            tp[:, i * 80:(i + 1) * 80],
                            b2[:, t_ * 128:(t_ + 1) * 128],
                            ident[0:M * K, 0:M * K])
                    nc.scalar.copy(
                        bpx[:, done:done + grp, :, :].rearrange("p t m k -> p (t m k)"),
                        tp[:, 0:80 * grp])
                    done += grp

                # ---- stage B: per-site bit stats (pixel layout) ----
                TM = NT * M
                dt_ = kp.tile([128, TM, K], F32, tag="dt")
                eq = kp.tile([128, TM, K], F32, tag="eq")
                s1 = kp.tile([128, TM, K], F32, tag="s1")
                dm = kp.tile([128, TM, K], F32, tag="dm")

                base = mp.tile([128, TM], F32, tag="base")
                k1t = mp.tile([128, TM], F32, tag="k1t")
                p2a = mp.tile([128, TM], F32, tag="p2a")
                bat = mp.tile([128, TM], F32, tag="bat")
                k2t = mp.tile([128, TM], F32, tag="k2t")
                p2b = mp.tile([128, TM], F32, tag="p2b")
                bbt = mp.tile([128, TM], F32, tag="bbt")
                mred = mp.tile([128, TM], F32, tag="mred")

                bpx3 = bpx[:].rearrange("p t m k -> p (t m) k")
                bpxF = bpx[:].rearrange("p t m k -> p (t m k)")

                def fl(t):  # [128,TM,K] -> flat 2D
                    return t[:].rearrange("p s k -> p (s k)")

                def bcm(t):  # [128,TM] -> broadcast over K
                    return t[:].unsqueeze(-1).broadcast_to([128, TM, K])

                nc.scalar.activation(fl(dt_), bpxF, ACT.Abs, bias=pconst[:, 30:31], scale=1.0)
                nc.scalar.activation(fl(s1), bpxF, ACT.Sign, bias=pconst[:, 30:31], scale=1.0)
                nc.scalar.activation(fl(s1), fl(s1), ACT.Relu)       # h
                nc.vector.tensor_mul(eq[:], s1[:], pow2b)
                nc.vector.tensor_reduce(base[:], eq[:], mybir.AxisListType.X, ALU.add)
                # first ambiguous bit
                nc.vector.tensor_reduce(mred[:], dt_[:], mybir.AxisListType.X, ALU.min)
                nc.vector.tensor_tensor(eq[:], dt_[:], bcm(mred), ALU.is_equal)
                nc.vector.scalar_tensor_tensor(s1[:], eq[:], -16.0, iota16b,
                                               ALU.mult, ALU.add)
                nc.vector.tensor_reduce(k1t[:], s1[:], mybir.AxisListType.X, ALU.min)
                nc.vector.tensor_tensor(eq[:], iotab, bcm(k1t), ALU.is_equal)
                nc.vector.tensor_mul(s1[:], eq[:], pow2b)
                nc.vector.tensor_reduce(p2a[:], s1[:], mybir.AxisListType.X, ALU.add)
                nc.vector.tensor_mul(s1[:], eq[:], bpx3)
                nc.vector.tensor_reduce(bat[:], s1[:], mybir.AxisListType.X, ALU.add)
                # second ambiguous bit
                nc.vector.scalar_tensor_tensor(dm[:], eq[:], 8.0, dt_[:],
                                               ALU.mult, ALU.add)
                nc.vector.tensor_reduce(mred[:], dm[:], mybir.AxisListType.X, ALU.min)
                nc.vector.tensor_tensor(eq[:], dm[:], bcm(mred), ALU.is_equal)
                nc.vector.scalar_tensor_tensor(s1[:], eq[:], -16.0, iota16b,
                                               ALU.mult, ALU.add)
                nc.vector.tensor_reduce(k2t[:], s1[:], mybir.AxisListType.X, ALU.min)
                nc.vector.tensor_tensor(eq[:], iotab, bcm(k2t), ALU.is_equal)
                nc.vector.tensor_mul(s1[:], eq[:], pow2b)
                nc.vector.tensor_reduce(p2b[:], s1[:], mybir.AxisListType.X, ALU.add)
                nc.vector.tensor_mul(s1[:], eq[:], bpx3)
                nc.vector.tensor_reduce(bbt[:], s1[:], mybir.AxisListType.X, ALU.add)

                # ---- stage C: pair/word/weight math ([128, NT*M]) ----
                def mt(tag):
                    return msc.tile([128, NT * M], F32, tag=tag, name=tag)

                klo, khi = mt("klo"), mt("khi")
                p2lo, p2hi = mt("p2lo"), mt("p2hi")
                blo, bhi = mt("blo"), mt("bhi")
                sc1, sc2 = mt("sc1"), mt("sc2")
                nc.vector.tensor_tensor(klo[:], k1t[:], k2t[:], ALU.min)
                nc.vector.tensor_tensor(khi[:], k1t[:], k2t[:], ALU.max)
                nc.vector.tensor_tensor(p2lo[:], p2a[:], p2b[:], ALU.min)
                nc.vector.tensor_tensor(p2hi[:], p2a[:], p2b[:], ALU.max)
                nc.vector.tensor_tensor(sc1[:], k1t[:], k2t[:], ALU.is_gt)   # swap
                nc.vector.tensor_tensor(sc2[:], bbt[:], bat[:], ALU.subtract)
                nc.vector.tensor_mul(sc1[:], sc1[:], sc2[:])
                nc.vector.tensor_tensor(blo[:], bat[:], sc1[:], ALU.add)
                nc.vector.tensor_tensor(sc2[:], bat[:], bbt[:], ALU.add)
                nc.vector.tensor_tensor(bhi[:], sc2[:], blo[:], ALU.subtract)
                # pid = khi*(khi-1)/2 + klo ; gidx = pid*256 + base8
                nc.vector.scalar_tensor_tensor(sc1[:], khi[:], -1.0, khi[:],
                                               ALU.add, ALU.mult)
                nc.vector.scalar_tensor_tensor(sc1[:], sc1[:], 0.5, klo[:],
                                               ALU.mult, ALU.add)            # pid
                # base_clear
                nc.vector.tensor_scalar(sc2[:], blo[:], 0.5, None, ALU.is_gt)
                nc.vector.tensor_mul(sc2[:], sc2[:], p2lo[:])
                nc.vector.tensor_tensor(base[:], base[:], sc2[:], ALU.subtract)
                nc.vector.tensor_scalar(sc2[:], bhi[:], 0.5, None, ALU.is_gt)
                nc.vector.tensor_mul(sc2[:], sc2[:], p2hi[:])
                nc.vector.tensor_tensor(base[:], base[:], sc2[:], ALU.subtract)
                # base8 = pack(base; p2lo, p2hi) -- int domain, mod 2^k = AND(2^k-1)
                I32 = mybir.dt.int32

                def mti(tag):
                    return msc.tile([128, NT * M], I32, tag=tag, name=tag)

                bci, loi, hii = mti("bci"), mti("loi"), mti("hii")
                t1i, t2i = mti("t1i"), mti("t2i")
                lowv = mt("lowv")
                nc.vector.tensor_copy(bci[:], base[:])
                nc.vector.tensor_copy(loi[:], p2lo[:])
                nc.vector.tensor_copy(hii[:], p2hi[:])
                nc.vector.tensor_scalar(loi[:], loi[:], -1, None, ALU.add)
                nc.vector.tensor_scalar(hii[:], hii[:], -1, None, ALU.add)
                nc.vector.tensor_tensor(loi[:], bci[:], loi[:], ALU.bitwise_and)
                nc.vector.tensor_tensor(hii[:], bci[:], hii[:], ALU.bitwise_and)
                nc.vector.tensor_tensor(t1i[:], hii[:], loi[:], ALU.subtract)
                nc.vector.tensor_scalar(t1i[:], t1i[:], 1, None, ALU.arith_shift_right)
                nc.vector.tensor_tensor(t2i[:], bci[:], hii[:], ALU.subtract)
                nc.vector.tensor_scalar(t2i[:], t2i[:], 2, None, ALU.arith_shift_right)
                nc.vector.tensor_tensor(t1i[:], t1i[:], t2i[:], ALU.add)
                nc.vector.tensor_tensor(t1i[:], t1i[:], loi[:], ALU.add)     # base8 int
                nc.vector.tensor_copy(lowv[:], t1i[:])                       # base8 f32
                nc.vector.scalar_tensor_tensor(lowv[:], sc1[:], 256.0, lowv[:],
                                               ALU.mult, ALU.add)            # gidx
                gidx16 = wp.tile([128, NT * M], I16, tag="gidx16")
                nc.vector.tensor_copy(gidx16[:], lowv[:])
                # weights, m-major: [128, M, NT, 4]
                wt = wp.tile([128, M, NT, 4], F32, tag="wt")
                nc.vector.tensor_scalar(sc1[:], blo[:], -1.0, 1.0, ALU.mult, ALU.add)
                nc.vector.tensor_scalar(sc2[:], bhi[:], -1.0, 1.0, ALU.mult, ALU.add)

                def wslot(jj):  # [128, M, NT] view ordered as (t, m)
                    return wt[:, :, :, jj].rearrange("p m t -> p t m")

                def v3(t):  # [128,TM] -> [128, NT, M]
                    return t[:].rearrange("p (t m) -> p t m", t=NT, m=M)

                nc.vector.tensor_mul(wslot(0), v3(sc1), v3(sc2))
                nc.vector.tensor_mul(wslot(1), v3(blo), v3(sc2))
                nc.vector.tensor_mul(wslot(2), v3(sc1), v3(bhi))
                nc.vector.tensor_mul(wslot(3), v3(blo), v3(bhi))

                # ---- stage D: index reshuffle to [16, num_idxs/16] ----
                idxt = idxp.tile([128, M, 256], I16, tag="idxt")
                idxv = idxt[0:16, :, :].rearrange("p m (t f) -> p m t f", t=NT, f=8)
                g16v = gidx16[:].rearrange("p (t m) -> p t m", t=NT, m=M)
                for phi in range(8):
                    for m in range(M):
                        nc.sync.dma_start(idxv[:, m, :, phi],
                                          g16v[phi * 16:(phi + 1) * 16, :, m])
                # q7 cores read idxs through their own 16-partition groups:
                # replicate group 0 into groups 1..7
                for grp in range(1, 8):
                    nc.sync.dma_start(idxt[16 * grp:16 * (grp + 1), :, :],
                                      idxt[0:16, :, :])

                # ---- stage E: gather + weighted vote reduce ----
                for m in range(M):
                    v = vp.tile([128, NT, 4, D], F32, tag="v")
                    if os.environ.get("KBISECT") == "nogather":
                        nc.vector.memset(v[:].rearrange("p t j d -> p (t j d)"), 0)
                    else:
                        nc.gpsimd.dma_gather(
                            out_ap=v[:].rearrange("p t j d -> p t (j d)"),
                            in_ap=pt_p.ap()[m],
                            idxs_ap=idxt[:, m, :],
                            num_idxs=NPX,
                            num_idxs_reg=NPX,
                            elem_size=4 * D,
                            single_packet=False,
                            queue_num=m % 4,
                        )
                    wv = wvp.tile([128, NT, 4, D], F32, tag="wv")
                    wb = wt[:, m, :, :].rearrange("p t j -> p (t j)") \
                        .unsqueeze(-1).broadcast_to([128, NT * 4, D])
                    nc.vector.tensor_tensor(
                        wv[:].rearrange("p t j d -> p (t j) d"),
                        v[:].rearrange("p t j d -> p (t j) d"), wb, ALU.mult)
                    nc.vector.tensor_add(
                        wv[:, :, 0:2, :].rearrange("p t j d -> p t (j d)"),
                        wv[:, :, 0:2, :].rearrange("p t j d -> p t (j d)"),
                        wv[:, :, 2:4, :].rearrange("p t j d -> p t (j d)"))
                    nc.vector.tensor_add(wv[:, :, 0, :], wv[:, :, 0, :],
                                         wv[:, :, 1, :])
                    if m == 0:
                        nc.scalar.copy(feat[:, img], wv[:, :, 0, :])
                    else:
                        nc.vector.tensor_add(feat[:, img], feat[:, img],
                                             wv[:, :, 0, :])

                # ---- stage F: 2x2 avg pool via PE ----
                pps_t = pps.tile([128, 8, D], F32, tag="pp")
                for g in range(8):
                    for s in range(4):
                        nc.tensor.matmul(
                            pps_t[:, g, :],
                            poolw[:, s * 128:(s + 1) * 128],
                            feat[:, img, 4 * g + s, :],
                            start=(s == 0), stop=(s == 3))
                nc.scalar.copy(
                    flatbuf[:, :, :, img],
                    pps_t[:].rearrange("p g d -> p d g"))

            # ---- classifier ----
            lg = lps.tile([NCLS, NI], F32)
            wqv = wqt[:].rearrange("p (c l) -> p c l", c=D * 8, l=NCLS)
            flv = flatbuf[:].rearrange("p d g i -> p (d g) i")
            for c_ in range(D * 8):
                nc.tensor.matmul(lg[:], wqv[:, c_, :], flv[:, c_, :],
                                 start=(c_ == 0), stop=(c_ == D * 8 - 1))
            lsb = flp.tile([NCLS, NI], F32)
            nc.scalar.activation(lsb[:], lg[:], ACT.Identity, bias=bpred[:], scale=1.0)
            nc.sync.dma_start(out_p.ap(), lsb[:])

    nc.compile()
    return nc


_CACHE: dict = {}


def _get_kernel(c1, c2, dy1, dx1, dy2, dx2):
    key = (c1.tobytes(), c2.tobytes(), dy1.tobytes(), dx1.tobytes(),
           dy2.tobytes(), dx2.tobytes())
    if key not in _CACHE:
        _CACHE[key] = _build_kernel(c1, c2, dy1, dx1, dy2, dx2)
    return _CACHE[key]


def kernel(x, c1, c2, dy1, dx1, dy2, dx2, thresholds, table, w_pred, b_pred):
    x = np.asarray(x, dtype=np.float32)
    c1, c2 = np.asarray(c1, np.int32), np.asarray(c2, np.int32)
    dy1, dx1 = np.asarray(dy1, np.int32), np.asarray(dx1, np.int32)
    dy2, dx2 = np.asarray(dy2, np.int32), np.asarray(dx2, np.int32)
    thresholds = np.asarray(thresholds, np.float32)
    table = np.asarray(table, np.float32)
    w_pred = np.asarray(w_pred, np.float32)
    b_pred = np.asarray(b_pred, np.float32)

    nc = _get_kernel(c1, c2, dy1, dx1, dy2, dx2)

    xp = np.pad(x, ((0, 0), (0, 0), (0, L - 1), (0, L - 1)))
    PT = _build_pair_table(table)
    pconst, thrneg, ident, poolW, wqT, bpred = _host_consts(thresholds, w_pred, b_pred)

    in_maps = []
    for c in range(NCORES):
        in_maps.append(dict(
            xp=np.ascontiguousarray(xp[c * NI:(c + 1) * NI]),
            pt=PT, pconst=pconst, thrneg=thrneg, ident=ident,
            poolw=poolW, wqt=wqT, bpred=bpred,
        ))
    res = run_bass_kernel_spmd(nc, in_maps, core_ids=list(range(NCORES)))
    outs = [r["out"].T for r in res.results]      # each [NI, NCLS]
    return np.concatenate(outs, axis=0).astype(np.float32)



# revision 19
# speedup vs baseline: 1.1970x; 1.1970x over previous
"""Trainium2 Bass kernel for the fern/sparse-table CTE model.

Strategy: data-parallel over batch N=32 across 8 cores (4 images each).
Stage A loads all 108 possible (c,dy,dx) windows with one overlapping-window
DMA per plane, then a PE matmul against a +1/-1 selection matrix (with the
threshold folded in via an accumulated ones-row matmul) produces thresholded
pixel-pair differences directly in transposed (pixel-major) layout; sigmoid
on the Scalar engine gives soft bits. Per-site top-2 ambiguous-bit extraction
runs on DVE; the T=4 table-row gather is served by dma_gather from a
host-prebuilt "pair table" whose 256B elements hold the 4 rows for a packed
(pair-id, base8) index. Gathers for the 8 ferns round-robin over the 4 SWDGE
queues so up to 4 descriptor-generation kernels run concurrently on distinct
Q7 core pairs. Votes are weighted and reduced on DVE; 2x2 average pooling and
the classifier run as PE matmuls.
"""
import os
import numpy as np
from contextlib import ExitStack

import concourse.bacc as bacc
import concourse.bass as bass
import concourse.tile as tile
from concourse import mybir
from concourse.bass_utils import run_bass_kernel_spmd

F32 = mybir.dt.float32
I16 = mybir.dt.int16
I32 = mybir.dt.int32
ALU = mybir.AluOpType
ACT = mybir.ActivationFunctionType

M, K, L = 8, 10, 6
D = 16                      # D_OUT
NCLS = 10
N, C, H, W = 32, 3, 64, 64
NCORES = 8
NI = N // NCORES            # images per core
HP = H + L - 1              # 69 padded
NPX = H * W                 # 4096
NT = NPX // 128             # 32 pixel tiles per image
NPAIR = 45
PTROWS = NPAIR * 256        # 11520 elements per fern
NWIN = C * L * L            # 108 distinct windows


def _build_pair_table(table: np.ndarray) -> np.ndarray:
    """PT[m, pid*256+base8, 64] f32; rows j=ilo+2*ihi of the 256B element are
    table[m*1024 + unpack(base8;klo,khi) + ilo*2^klo + ihi*2^khi]."""
    tbl = table.reshape(M, 1024, D)
    PT = np.zeros((M, PTROWS, 4 * D), dtype=np.float32)
    base8 = np.arange(256)
    for khi in range(K):
        for klo in range(khi):
            pid = khi * (khi - 1) // 2 + klo
            rest = [k for k in range(K) if k not in (klo, khi)]
            unpacked = np.zeros(256, dtype=np.int64)
            for r, k in enumerate(rest):
                unpacked += ((base8 >> r) & 1) << k
            for ihi in range(2):
                for ilo in range(2):
                    j = ilo + 2 * ihi
                    rows = unpacked + ilo * (1 << klo) + ihi * (1 << khi)
                    PT[:, pid * 256 + base8, j * D:(j + 1) * D] = tbl[:, rows, :]
    return PT


def _host_consts(thresholds, w_pred, b_pred, c1, c2, dy1, dx1, dy2, dx2):
    pconst = np.zeros((128, 32), dtype=np.float32)
    pconst[:, 30] = -0.5
    pconst[:, 0:10] = (1 << np.arange(K)).astype(np.float32)[None, :]
    pconst[:, 10:20] = np.arange(K, dtype=np.float32)[None, :]
    pconst[:, 20:30] = np.arange(K, dtype=np.float32)[None, :] + 16.0
    # window-selection matrix with thresholds in row 108 (paired with a ones
    # lhsT row): z[site, r] = win1_r[site] - win2_r[site] - thr_r
    pw = np.zeros((128, M * K), dtype=np.float32)
    w1 = (np.asarray(c1) * 36 + np.asarray(dy1) * 6 + np.asarray(dx1)).reshape(-1)
    w2 = (np.asarray(c2) * 36 + np.asarray(dy2) * 6 + np.asarray(dx2)).reshape(-1)
    for r in range(M * K):
        pw[w1[r], r] += 1.0
        pw[w2[r], r] -= 1.0
    onesc = np.ones((1, 128 + M * K), dtype=np.float32)
    onesc[0, 128:] = -thresholds.reshape(-1)
    # pool lhsT: poolW[p, s*128 + s2*32 + w2] = 0.25 if s2==s and (p%64)//2==w2
    poolW = np.zeros((128, 4, 4, 32), dtype=np.float32)
    p = np.arange(128)
    for s in range(4):
        poolW[p, s, s, (p % 64) // 2] = 0.25
    poolW = poolW.reshape(128, 512)
    # classifier lhsT: wqT[p, c, cls] = w_pred[cls, d*1024 + (4g+s)*32 + w2]
    # with c = d*8+g, p = s*32+w2
    wq = w_pred.reshape(NCLS, D, 8, 4, 32)          # [cls, d, g, s, w2]
    wqT = np.transpose(wq, (3, 4, 1, 2, 0)).reshape(128, D * 8, NCLS)
    wqT = np.ascontiguousarray(wqT.reshape(128, D * 8 * NCLS)).astype(np.float32)
    bpred = b_pred.reshape(NCLS, 1).astype(np.float32)
    return pconst, pw, onesc, poolW, wqT, bpred


def _build_kernel(c1, c2, dy1, dx1, dy2, dx2):
    """Build + compile the per-core kernel. Fern geometry is baked into the
    selection matrix at trace time."""
    nc = bacc.Bacc("TRN2", num_devices=NCORES, num_swdge_queues=4)

    xp_p = nc.declare_dram_parameter("xp", [NI, C, HP, HP], F32, isOutput=False)
    pt_p = nc.declare_dram_parameter("pt", [M, PTROWS, 4 * D], F32, isOutput=False)
    pc_p = nc.declare_dram_parameter("pconst", [128, 32], F32, isOutput=False)
    pw_p = nc.declare_dram_parameter("pwsel", [128, M * K], F32, isOutput=False)
    on_p = nc.declare_dram_parameter("onesc", [1, 128 + M * K], F32, isOutput=False)
    pl_p = nc.declare_dram_parameter("poolw", [128, 512], F32, isOutput=False)
    wq_p = nc.declare_dram_parameter("wqt", [128, D * 8 * NCLS], F32, isOutput=False)
    bp_p = nc.declare_dram_parameter("bpred", [NCLS, 1], F32, isOutput=False)
    out_p = nc.declare_dram_parameter("out", [NCLS, NI], F32, isOutput=True)

    with tile.TileContext(nc, num_cores=NCORES) as tc:
        with ExitStack() as ctx:
            cpool = ctx.enter_context(tc.tile_pool(name="consts", bufs=1))
            awp = ctx.enter_context(tc.tile_pool(name="allwin", bufs=2))
            bpxp = ctx.enter_context(tc.tile_pool(name="bpx", bufs=2))
            kp = ctx.enter_context(tc.tile_pool(name="kstage", bufs=1))
            mp = ctx.enter_context(tc.tile_pool(name="mstage", bufs=1))
            msc = ctx.enter_context(tc.tile_pool(name="mscratch", bufs=1))
            wp = ctx.enter_context(tc.tile_pool(name="wgidx", bufs=2))
            idxp = ctx.enter_context(tc.tile_pool(name="idx", bufs=2))
            vp = ctx.enter_context(tc.tile_pool(name="votes", bufs=5))
            wvp = ctx.enter_context(tc.tile_pool(name="wv", bufs=1))
            fp = ctx.enter_context(tc.tile_pool(name="feat", bufs=1))
            flp = ctx.enter_context(tc.tile_pool(name="flat", bufs=1))
            tps = ctx.enter_context(tc.tile_pool(name="tpsum", bufs=3, space="PSUM"))
            pps = ctx.enter_context(tc.tile_pool(name="ppsum", bufs=2, space="PSUM"))
            lps = ctx.enter_context(tc.tile_pool(name="lpsum", bufs=1, space="PSUM"))

            # ---- constants ----
            pconst = cpool.tile([128, 32], F32)
            nc.sync.dma_start(pconst[:], pc_p.ap())
            pwsel = cpool.tile([128, M * K], F32)
            nc.sync.dma_start(pwsel[:], pw_p.ap())
            onesc = cpool.tile([1, 128 + M * K], F32)
            nc.sync.dma_start(onesc[:], on_p.ap())
            poolw = cpool.tile([128, 512], F32)
            nc.sync.dma_start(poolw[:], pl_p.ap())
            wqt = cpool.tile([128, D * 8 * NCLS], F32)
            nc.sync.dma_start(wqt[:], wq_p.ap())
            bpred = cpool.tile([NCLS, 1], F32)
            nc.sync.dma_start(bpred[:], bp_p.ap())

            def bc10(col):  # [128,10] const col -> [128,NT*M,10] broadcast
                v = pconst[:, col:col + 10]
                return v.unsqueeze(1).broadcast_to([128, NT * M, K])

            pow2b, iotab, iota16b = bc10(0), bc10(10), bc10(20)

            feat = fp.tile([128, NI, NT, D], F32)
            flatbuf = flp.tile([128, D, 8, NI], F32)

            for img in range(NI):
                # ---- stage A: all-windows load + select/transpose matmul ----
                allwin = awp.tile([128, NPX], F32, tag="allwin")
                for c in range(C):
                    for dy in range(L):
                        src = bass.AP(xp_p.ap().tensor,
                                      (img * C + c) * HP * HP + dy * HP,
                                      [[1, L], [HP, H], [1, W]])
                        w0 = c * 36 + dy * 6
                        nc.sync.dma_start(
                            allwin[w0:w0 + 6, :]
                            .rearrange("p (a b) -> p a b", a=H, b=W), src)

                bpx = bpxp.tile([128, NT, M, K], F32, tag="bpx")
                done = 0
                while done < NT:
                    grp = min(4, NT - done)
                    tp = tps.tile([128, 4 * M * K], F32, tag="tp")
                    for i in range(grp):
                        t_ = done + i
                        po = tp[:, i * 80:(i + 1) * 80]
                        nc.tensor.matmul(
                            po, allwin[0:NWIN, t_ * 128:(t_ + 1) * 128],
                            pwsel[0:NWIN, :], start=True, stop=False)
                        nc.tensor.matmul(
                            po, onesc[0:1, 0:128],
                            onesc[0:1, 128:128 + M * K], start=False, stop=True)
                    nc.scalar.activation(
                        bpx[:, done:done + grp, :, :]
                        .rearrange("p t m k -> p (t m k)"),
                        tp[:, 0:80 * grp], ACT.Sigmoid)
                    done += grp

                # ---- stage B: per-site bit stats (pixel layout) ----
                TM = NT * M
                dt_ = kp.tile([128, TM, K], F32, tag="dt")
                eq = kp.tile([128, TM, K], F32, tag="eq")
                s1 = kp.tile([128, TM, K], F32, tag="s1")
                dm = kp.tile([128, TM, K], F32, tag="dm")

                base = mp.tile([128, TM], F32, tag="base")
                k1t = mp.tile([128, TM], F32, tag="k1t")
                p2a = mp.tile([128, TM], F32, tag="p2a")
                bat = mp.tile([128, TM], F32, tag="bat")
                k2t = mp.tile([128, TM], F32, tag="k2t")
                p2b = mp.tile([128, TM], F32, tag="p2b")
                bbt = mp.tile([128, TM], F32, tag="bbt")
                mred = mp.tile([128, TM], F32, tag="mred")

                bpx3 = bpx[:].rearrange("p t m k -> p (t m) k")
                bpxF = bpx[:].rearrange("p t m k -> p (t m k)")

                def fl(t):  # [128,TM,K] -> flat 2D
                    return t[:].rearrange("p s k -> p (s k)")

                def bcm(t):  # [128,TM] -> broadcast over K
                    return t[:].unsqueeze(-1).broadcast_to([128, TM, K])

                nc.scalar.activation(fl(dt_), bpxF, ACT.Abs, bias=pconst[:, 30:31], scale=1.0)
                nc.scalar.activation(fl(s1), bpxF, ACT.Sign, bias=pconst[:, 30:31], scale=1.0)
                nc.scalar.activation(fl(s1), fl(s1), ACT.Relu)       # h
                nc.vector.tensor_mul(eq[:], s1[:], pow2b)
                nc.vector.tensor_reduce(base[:], eq[:], mybir.AxisListType.X, ALU.add)
                # first ambiguous bit
                nc.vector.tensor_reduce(mred[:], dt_[:], mybir.AxisListType.X, ALU.min)
                nc.vector.tensor_tensor(eq[:], dt_[:], bcm(mred), ALU.is_equal)
                nc.vector.scalar_tensor_tensor(s1[:], eq[:], -16.0, iota16b,
                                               ALU.mult, ALU.add)
                nc.vector.tensor_reduce(k1t[:], s1[:], mybir.AxisListType.X, ALU.min)
                nc.vector.tensor_tensor(eq[:], iotab, bcm(k1t), ALU.is_equal)
                nc.vector.tensor_mul(s1[:], eq[:], pow2b)
                nc.vector.tensor_reduce(p2a[:], s1[:], mybir.AxisListType.X, ALU.add)
                nc.vector.tensor_mul(s1[:], eq[:], bpx3)
                nc.vector.tensor_reduce(bat[:], s1[:], mybir.AxisListType.X, ALU.add)
                # second ambiguous bit
                nc.vector.scalar_tensor_tensor(dm[:], eq[:], 8.0, dt_[:],
                                               ALU.mult, ALU.add)
                nc.vector.tensor_reduce(mred[:], dm[:], mybir.AxisListType.X, ALU.min)
                nc.vector.tensor_tensor(eq[:], dm[:], bcm(mred), ALU.is_equal)
                nc.vector.scalar_tensor_tensor(s1[:], eq[:], -16.0, iota16b,
                                               ALU.mult, ALU.add)
                nc.vector.tensor_reduce(k2t[:], s1[:], mybir.AxisListType.X, ALU.min)
                nc.vector.tensor_tensor(eq[:], iotab, bcm(k2t), ALU.is_equal)
                nc.vector.tensor_mul(s1[:], eq[:], pow2b)
                nc.vector.tensor_reduce(p2b[:], s1[:], mybir.AxisListType.X, ALU.add)
                nc.vector.tensor_mul(s1[:], eq[:], bpx3)
                nc.vector.tensor_reduce(bbt[:], s1[:], mybir.AxisListType.X, ALU.add)

                # ---- stage C: pair/word/weight math ([128, NT*M]) ----
                def mt(tag):
                    return msc.tile([128, NT * M], F32, tag=tag, name=tag)

                klo, khi = mt("klo"), mt("khi")
                p2lo, p2hi = mt("p2lo"), mt("p2hi")
                blo, bhi = mt("blo"), mt("bhi")
                sc1, sc2 = mt("sc1"), mt("sc2")
                nc.vector.tensor_tensor(klo[:], k1t[:], k2t[:], ALU.min)
                nc.vector.tensor_tensor(khi[:], k1t[:], k2t[:], ALU.max)
                nc.vector.tensor_tensor(p2lo[:], p2a[:], p2b[:], ALU.min)
                nc.vector.tensor_tensor(p2hi[:], p2a[:], p2b[:], ALU.max)
                nc.vector.tensor_tensor(sc1[:], k1t[:], k2t[:], ALU.is_gt)   # swap
                nc.vector.tensor_tensor(sc2[:], bbt[:], bat[:], ALU.subtract)
                nc.vector.tensor_mul(sc1[:], sc1[:], sc2[:])
                nc.vector.tensor_tensor(blo[:], bat[:], sc1[:], ALU.add)
                nc.vector.tensor_tensor(sc2[:], bat[:], bbt[:], ALU.add)
                nc.vector.tensor_tensor(bhi[:], sc2[:], blo[:], ALU.subtract)
                # pid = khi*(khi-1)/2 + klo ; gidx = pid*256 + base8
                nc.vector.scalar_tensor_tensor(sc1[:], khi[:], -1.0, khi[:],
                                               ALU.add, ALU.mult)
                nc.vector.scalar_tensor_tensor(sc1[:], sc1[:], 0.5, klo[:],
                                               ALU.mult, ALU.add)            # pid
                # base_clear
                nc.vector.tensor_scalar(sc2[:], blo[:], 0.5, None, ALU.is_gt)
                nc.vector.tensor_mul(sc2[:], sc2[:], p2lo[:])
                nc.vector.tensor_tensor(base[:], base[:], sc2[:], ALU.subtract)
                nc.vector.tensor_scalar(sc2[:], bhi[:], 0.5, None, ALU.is_gt)
                nc.vector.tensor_mul(sc2[:], sc2[:], p2hi[:])
                nc.vector.tensor_tensor(base[:], base[:], sc2[:], ALU.subtract)
                # base8 = pack(base; p2lo, p2hi) -- int domain, mod 2^k = AND(2^k-1)

                def mti(tag):
                    return msc.tile([128, NT * M], I32, tag=tag, name=tag)

                bci, loi, hii = mti("bci"), mti("loi"), mti("hii")
                t1i, t2i = mti("t1i"), mti("t2i")
                lowv = mt("lowv")
                nc.vector.tensor_copy(bci[:], base[:])
                nc.vector.tensor_copy(loi[:], p2lo[:])
                nc.vector.tensor_copy(hii[:], p2hi[:])
                nc.vector.tensor_scalar(loi[:], loi[:], -1, None, ALU.add)
                nc.vector.tensor_scalar(hii[:], hii[:], -1, None, ALU.add)
                nc.vector.tensor_tensor(loi[:], bci[:], loi[:], ALU.bitwise_and)
                nc.vector.tensor_tensor(hii[:], bci[:], hii[:], ALU.bitwise_and)
                nc.vector.tensor_tensor(t1i[:], hii[:], loi[:], ALU.subtract)
                nc.vector.tensor_scalar(t1i[:], t1i[:], 1, None, ALU.arith_shift_right)
                nc.vector.tensor_tensor(t2i[:], bci[:], hii[:], ALU.subtract)
                nc.vector.tensor_scalar(t2i[:], t2i[:], 2, None, ALU.arith_shift_right)
                nc.vector.tensor_tensor(t1i[:], t1i[:], t2i[:], ALU.add)
                nc.vector.tensor_tensor(t1i[:], t1i[:], loi[:], ALU.add)     # base8 int
                nc.vector.tensor_copy(lowv[:], t1i[:])                       # base8 f32
                nc.vector.scalar_tensor_tensor(lowv[:], sc1[:], 256.0, lowv[:],
                                               ALU.mult, ALU.add)            # gidx
                gidx16 = wp.tile([128, NT * M], I16, tag="gidx16")
                nc.vector.tensor_copy(gidx16[:], lowv[:])
                # weights, m-major: [128, M, NT, 4]
                wt = wp.tile([128, M, NT, 4], F32, tag="wt")
                nc.vector.tensor_scalar(sc1[:], blo[:], -1.0, 1.0, ALU.mult, ALU.add)
                nc.vector.tensor_scalar(sc2[:], bhi[:], -1.0, 1.0, ALU.mult, ALU.add)

                def wslot(jj):  # [128, M, NT] view ordered as (t, m)
                    return wt[:, :, :, jj].rearrange("p m t -> p t m")

                def v3(t):  # [128,TM] -> [128, NT, M]
                    return t[:].rearrange("p (t m) -> p t m", t=NT, m=M)

                nc.vector.tensor_mul(wslot(0), v3(sc1), v3(sc2))
                nc.vector.tensor_mul(wslot(1), v3(blo), v3(sc2))
                nc.vector.tensor_mul(wslot(2), v3(sc1), v3(bhi))
                nc.vector.tensor_mul(wslot(3), v3(blo), v3(bhi))

                # ---- stage D: wrap idxs to [16, n/16] layout, replicated ----
                idxs2 = idxp.tile([16, NT * M, 8], I16, tag="idxs2")
                for f in range(8):
                    nc.scalar.dma_start(idxs2[:, :, f],
                                        gidx16[f * 16:(f + 1) * 16, :])
                idxt = idxp.tile([128, M, NT, 8], I16, tag="idxt")
                i2v = idxs2[:].rearrange("q (t m) f -> q m t f", t=NT, m=M)
                for m in range(M):
                    nc.scalar.dma_start(idxt[0:16, m, :, :], i2v[:, m])
                for g in range(1, 8):
                    nc.scalar.dma_start(idxt[16 * g:16 * (g + 1), :, :, :],
                                        idxt[0:16, :, :, :])

                # ---- stage E: gather + weighted vote reduce ----
                for m in range(M):
                    v = vp.tile([128, NT, 4, D], F32, tag="v")
                    if os.environ.get("KBISECT") == "nogather":
                        nc.vector.memset(v[:].rearrange("p t j d -> p (t j d)"), 0)
                    else:
                        nc.gpsimd.dma_gather(
                            out_ap=v[:].rearrange("p t j d -> p t (j d)"),
                            in_ap=pt_p.ap()[m],
                            idxs_ap=idxt[:, m, :, :],
                            num_idxs=NPX,
                            num_idxs_reg=NPX,
                            elem_size=4 * D,
                            single_packet=False,
                            queue_num=m % 4,
                        )
                    wv = wvp.tile([128, NT, 4, D], F32, tag="wv")
                    wb = wt[:, m, :, :].rearrange("p t j -> p (t j)") \
                        .unsqueeze(-1).broadcast_to([128, NT * 4, D])
                    nc.vector.tensor_tensor(
                        wv[:].rearrange("p t j d -> p (t j) d"),
                        v[:].rearrange("p t j d -> p (t j) d"), wb, ALU.mult)
                    nc.vector.tensor_add(
                        wv[:, :, 0:2, :].rearrange("p t j d -> p t (j d)"),
                        wv[:, :, 0:2, :].rearrange("p t j d -> p t (j d)"),
                        wv[:, :, 2:4, :].rearrange("p t j d -> p t (j d)"))
                    nc.vector.tensor_add(wv[:, :, 0, :], wv[:, :, 0, :],
                                         wv[:, :, 1, :])
                    if m == 0:
                        nc.scalar.copy(feat[:, img], wv[:, :, 0, :])
                    else:
                        nc.vector.tensor_add(feat[:, img], feat[:, img],
                                             wv[:, :, 0, :])

                # ---- stage F: 2x2 avg pool via PE ----
                pps_t = pps.tile([128, 8, D], F32, tag="pp")
                for g in range(8):
                    for s in range(4):
                        nc.tensor.matmul(
                            pps_t[:, g, :],
                            poolw[:, s * 128:(s + 1) * 128],
                            feat[:, img, 4 * g + s, :],
                            start=(s == 0), stop=(s == 3))
                nc.scalar.copy(
                    flatbuf[:, :, :, img],
                    pps_t[:].rearrange("p g d -> p d g"))

            # ---- classifier ----
            lg = lps.tile([NCLS, NI], F32)
            wqv = wqt[:].rearrange("p (c l) -> p c l", c=D * 8, l=NCLS)
            flv = flatbuf[:].rearrange("p d g i -> p (d g) i")
            for c_ in range(D * 8):
                nc.tensor.matmul(lg[:], wqv[:, c_, :], flv[:, c_, :],
                                 start=(c_ == 0), stop=(c_ == D * 8 - 1))
            lsb = flp.tile([NCLS, NI], F32)
            nc.scalar.activation(lsb[:], lg[:], ACT.Identity, bias=bpred[:], scale=1.0)
            nc.sync.dma_start(out_p.ap(), lsb[:])

    nc.compile()
    return nc


_CACHE: dict = {}


def _get_kernel(c1, c2, dy1, dx1, dy2, dx2):
    key = (c1.tobytes(), c2.tobytes(), dy1.tobytes(), dx1.tobytes(),
           dy2.tobytes(), dx2.tobytes())
    if key not in _CACHE:
        _CACHE[key] = _build_kernel(c1, c2, dy1, dx1, dy2, dx2)
    return _CACHE[key]


def kernel(x, c1, c2, dy1, dx1, dy2, dx2, thresholds, table, w_pred, b_pred):
    x = np.asarray(x, dtype=np.float32)
    c1, c2 = np.asarray(c1, np.int32), np.asarray(c2, np.int32)
    dy1, dx1 = np.asarray(dy1, np.int32), np.asarray(dx1, np.int32)
    dy2, dx2 = np.asarray(dy2, np.int32), np.asarray(dx2, np.int32)
    thresholds = np.asarray(thresholds, np.float32)
    table = np.asarray(table, np.float32)
    w_pred = np.asarray(w_pred, np.float32)
    b_pred = np.asarray(b_pred, np.float32)

    nc = _get_kernel(c1, c2, dy1, dx1, dy2, dx2)

    xp = np.pad(x, ((0, 0), (0, 0), (0, L - 1), (0, L - 1)))
    PT = _build_pair_table(table)
    pconst, pwsel, onesc, poolW, wqT, bpred = _host_consts(
        thresholds, w_pred, b_pred, c1, c2, dy1, dx1, dy2, dx2)

    in_maps = []
    for c in range(NCORES):
        in_maps.append(dict(
            xp=np.ascontiguousarray(xp[c * NI:(c + 1) * NI]),
            pt=PT, pconst=pconst, pwsel=pwsel, onesc=onesc,
            poolw=poolW, wqt=wqT, bpred=bpred,
        ))
    res = run_bass_kernel_spmd(nc, in_maps, core_ids=list(range(NCORES)))
    outs = [r["out"].T for r in res.results]      # each [NI, NCLS]
    return np.concatenate(outs, axis=0).astype(np.float32)


# revision 24
# speedup vs baseline: 1.7781x; 1.4855x over previous
"""Trainium2 Bass kernel for the fern/sparse-table CTE model.

Strategy: data-parallel over batch N=32 across 8 cores (4 images each).
Stage A loads all 108 possible (c,dy,dx) windows with one overlapping-window
DMA per plane, then a PE matmul against a +1/-1 selection matrix (with the
threshold folded in via an accumulated ones-row matmul) produces thresholded
pixel-pair differences directly in transposed (pixel-major) layout; sigmoid
on the Scalar engine gives soft bits. Per-site top-2 ambiguous-bit extraction
runs on DVE; the T=4 table-row gather is served by dma_gather from a
host-prebuilt "pair table" whose 256B elements hold the 4 rows for a packed
(pair-id, base8) index. Gathers for the 8 ferns round-robin over the 4 SWDGE
queues so up to 4 descriptor-generation kernels run concurrently on distinct
Q7 core pairs. Votes are weighted and reduced on DVE; 2x2 average pooling and
the classifier run as PE matmuls.
"""
import os
import numpy as np
from contextlib import ExitStack

import concourse.bacc as bacc
import concourse.bass as bass
import concourse.tile as tile
from concourse import mybir
from concourse.bass_utils import run_bass_kernel_spmd

F32 = mybir.dt.float32
I16 = mybir.dt.int16
I32 = mybir.dt.int32
ALU = mybir.AluOpType
ACT = mybir.ActivationFunctionType

M, K, L = 8, 10, 6
D = 16                      # D_OUT
NCLS = 10
N, C, H, W = 32, 3, 64, 64
NCORES = 8
NI = N // NCORES            # images per core
HP = H + L - 1              # 69 padded
NPX = H * W                 # 4096
NT = NPX // 128             # 32 pixel tiles per image
NPAIR = 45
PTROWS = NPAIR * 256        # 11520 elements per fern
NWIN = C * L * L            # 108 distinct windows


def _build_pair_table(table: np.ndarray) -> np.ndarray:
    """PT[m, pid*256+base8, 64] f32; rows j=ilo+2*ihi of the 256B element are
    table[m*1024 + unpack(base8;klo,khi) + ilo*2^klo + ihi*2^khi]."""
    tbl = table.reshape(M, 1024, D)
    PT = np.zeros((M, PTROWS, 4 * D), dtype=np.float32)
    base8 = np.arange(256)
    for khi in range(K):
        for klo in range(khi):
            pid = khi * (khi - 1) // 2 + klo
            rest = [k for k in range(K) if k not in (klo, khi)]
            unpacked = np.zeros(256, dtype=np.int64)
            for r, k in enumerate(rest):
                unpacked += ((base8 >> r) & 1) << k
            for ihi in range(2):
                for ilo in range(2):
                    j = ilo + 2 * ihi
                    rows = unpacked + ilo * (1 << klo) + ihi * (1 << khi)
                    PT[:, pid * 256 + base8, j * D:(j + 1) * D] = tbl[:, rows, :]
    return PT


def _host_consts(thresholds, w_pred, b_pred, c1, c2, dy1, dx1, dy2, dx2):
    pconst = np.zeros((128, 32), dtype=np.float32)
    pconst[:, 30] = -0.5
    pconst[:, 0:10] = (1 << np.arange(K)).astype(np.float32)[None, :]
    pconst[:, 10:20] = np.arange(K, dtype=np.float32)[None, :]
    pconst[:, 20:30] = np.arange(K, dtype=np.float32)[None, :] + 16.0
    # window-selection matrix with thresholds in row 108 (paired with a ones
    # lhsT row): z[site, r] = win1_r[site] - win2_r[site] - thr_r
    pw = np.zeros((128, M * K), dtype=np.float32)
    w1 = (np.asarray(c1) * 36 + np.asarray(dy1) * 6 + np.asarray(dx1)).reshape(-1)
    w2 = (np.asarray(c2) * 36 + np.asarray(dy2) * 6 + np.asarray(dx2)).reshape(-1)
    for r in range(M * K):
        pw[w1[r], r] += 1.0
        pw[w2[r], r] -= 1.0
    pw[NWIN, :] = -thresholds.reshape(-1)
    ones4k = np.ones((1, NPX), dtype=np.float32)
    ident = np.eye(128, dtype=np.float32)
    # pool lhsT: poolW[p, s*128 + s2*32 + w2] = 0.25 if s2==s and (p%64)//2==w2
    poolW = np.zeros((128, 4, 4, 32), dtype=np.float32)
    p = np.arange(128)
    for s in range(4):
        poolW[p, s, s, (p % 64) // 2] = 0.25
    poolW = poolW.reshape(128, 512)
    # classifier lhsT: wqT[p, c, cls] = w_pred[cls, d*1024 + (4g+s)*32 + w2]
    # with c = d*8+g, p = s*32+w2
    wq = w_pred.reshape(NCLS, D, 8, 4, 32)          # [cls, d, g, s, w2]
    wqT = np.transpose(wq, (3, 4, 1, 2, 0)).reshape(128, D * 8, NCLS)
    wqT = np.ascontiguousarray(wqT.reshape(128, D * 8 * NCLS)).astype(np.float32)
    bpred = b_pred.reshape(NCLS, 1).astype(np.float32)
    return pconst, pw, ones4k, ident, poolW, wqT, bpred


def _build_kernel(c1, c2, dy1, dx1, dy2, dx2):
    """Build + compile the per-core kernel. Fern geometry is baked into the
    selection matrix at trace time."""
    nc = bacc.Bacc("TRN2", num_devices=NCORES, num_swdge_queues=4)

    xp_p = nc.declare_dram_parameter("xp", [NI, C, HP, HP], F32, isOutput=False)
    pt_p = nc.declare_dram_parameter("pt", [M, PTROWS, 4 * D], F32, isOutput=False)
    pc_p = nc.declare_dram_parameter("pconst", [128, 32], F32, isOutput=False)
    pw_p = nc.declare_dram_parameter("pwsel", [128, M * K], F32, isOutput=False)
    on_p = nc.declare_dram_parameter("ones4k", [1, NPX], F32, isOutput=False)
    id_p = nc.declare_dram_parameter("ident", [128, 128], F32, isOutput=False)
    pl_p = nc.declare_dram_parameter("poolw", [128, 512], F32, isOutput=False)
    wq_p = nc.declare_dram_parameter("wqt", [128, D * 8 * NCLS], F32, isOutput=False)
    bp_p = nc.declare_dram_parameter("bpred", [NCLS, 1], F32, isOutput=False)
    out_p = nc.declare_dram_parameter("out", [NCLS, NI], F32, isOutput=True)

    with tile.TileContext(nc, num_cores=NCORES) as tc:
        with ExitStack() as ctx:
            cpool = ctx.enter_context(tc.tile_pool(name="consts", bufs=1))
            awp = ctx.enter_context(tc.tile_pool(name="allwin", bufs=2))
            bpxp = ctx.enter_context(tc.tile_pool(name="bpx", bufs=2))
            kp = ctx.enter_context(tc.tile_pool(name="kstage", bufs=1))
            mp = ctx.enter_context(tc.tile_pool(name="mstage", bufs=1))
            msc = ctx.enter_context(tc.tile_pool(name="mscratch", bufs=1))
            wp = ctx.enter_context(tc.tile_pool(name="wgidx", bufs=2))
            idxp = ctx.enter_context(tc.tile_pool(name="idx", bufs=2))
            vp = ctx.enter_context(tc.tile_pool(name="votes", bufs=5))
            wvp = ctx.enter_context(tc.tile_pool(name="wv", bufs=1))
            fp = ctx.enter_context(tc.tile_pool(name="feat", bufs=1))
            flp = ctx.enter_context(tc.tile_pool(name="flat", bufs=1))
            tps = ctx.enter_context(tc.tile_pool(name="tpsum", bufs=3, space="PSUM"))
            dps = ctx.enter_context(tc.tile_pool(name="dpsum", bufs=2, space="PSUM"))
            pps = ctx.enter_context(tc.tile_pool(name="ppsum", bufs=2, space="PSUM"))
            lps = ctx.enter_context(tc.tile_pool(name="lpsum", bufs=1, space="PSUM"))

            # ---- constants ----
            pconst = cpool.tile([128, 32], F32)
            nc.sync.dma_start(pconst[:], pc_p.ap())
            pwsel = cpool.tile([128, M * K], F32)
            nc.sync.dma_start(pwsel[:], pw_p.ap())
            ones4k = cpool.tile([1, NPX], F32)
            nc.sync.dma_start(ones4k[:], on_p.ap())
            ident = cpool.tile([128, 128], F32)
            nc.sync.dma_start(ident[:], id_p.ap())
            poolw = cpool.tile([128, 512], F32)
            nc.sync.dma_start(poolw[:], pl_p.ap())
            wqt = cpool.tile([128, D * 8 * NCLS], F32)
            nc.sync.dma_start(wqt[:], wq_p.ap())
            bpred = cpool.tile([NCLS, 1], F32)
            nc.sync.dma_start(bpred[:], bp_p.ap())

            def bc10(col):  # [128,10] const col -> [128,NT*M,10] broadcast
                v = pconst[:, col:col + 10]
                return v.unsqueeze(1).broadcast_to([128, NT * M, K])

            pow2b, iotab, iota16b = bc10(0), bc10(10), bc10(20)

            feat = fp.tile([128, NI, NT, D], F32)
            flatbuf = flp.tile([128, D, 8, NI], F32)

            bpx_tiles = [None] * NI

            def emit_stage_a(img):
                # ---- stage A: all-windows load + select/transpose matmul ----
                allwin = awp.tile([128, NPX], F32, tag="allwin")
                for c in range(C):
                    for dy in range(L):
                        src = bass.AP(xp_p.ap().tensor,
                                      (img * C + c) * HP * HP + dy * HP,
                                      [[1, L], [HP, H], [1, W]])
                        w0 = c * 36 + dy * 6
                        nc.sync.dma_start(
                            allwin[w0:w0 + 6, :]
                            .rearrange("p (a b) -> p a b", a=H, b=W), src)
                nc.sync.dma_start(allwin[NWIN:NWIN + 1, :], on_p.ap())

                bpx = bpxp.tile([128, NT, M, K], F32, tag="bpx")
                bpx_tiles[img] = bpx
                done = 0
                while done < NT:
                    grp = min(4, NT - done)
                    tp = tps.tile([128, 4 * M * K], F32, tag="tp")
                    for i in range(grp):
                        t_ = done + i
                        nc.tensor.matmul(
                            tp[:, i * 80:(i + 1) * 80],
                            allwin[0:NWIN + 1, t_ * 128:(t_ + 1) * 128],
                            pwsel[0:NWIN + 1, :], start=True, stop=True)
                    nc.scalar.activation(
                        bpx[:, done:done + grp, :, :]
                        .rearrange("p t m k -> p (t m k)"),
                        tp[:, 0:80 * grp], ACT.Sigmoid)
                    done += grp

            emit_stage_a(0)
            for img in range(NI):
                bpx = bpx_tiles[img]
                # ---- stage B: per-site bit stats (pixel layout) ----
                TM = NT * M
                dt_ = kp.tile([128, TM, K], F32, tag="dt")
                eq = kp.tile([128, TM, K], F32, tag="eq")
                s1 = kp.tile([128, TM, K], F32, tag="s1")
                dm = dt_

                base = mp.tile([128, TM], F32, tag="base")
                k1t = mp.tile([128, TM], F32, tag="k1t")
                p2a = mp.tile([128, TM], F32, tag="p2a")
                bat = mp.tile([128, TM], F32, tag="bat")
                k2t = mp.tile([128, TM], F32, tag="k2t")
                p2b = mp.tile([128, TM], F32, tag="p2b")
                bbt = mp.tile([128, TM], F32, tag="bbt")
                mred = mp.tile([128, TM], F32, tag="mred")

                bpx3 = bpx[:].rearrange("p t m k -> p (t m) k")
                bpxF = bpx[:].rearrange("p t m k -> p (t m k)")

                def fl(t):  # [128,TM,K] -> flat 2D
                    return t[:].rearrange("p s k -> p (s k)")

                def bcm(t):  # [128,TM] -> broadcast over K
                    return t[:].unsqueeze(-1).broadcast_to([128, TM, K])

                nc.scalar.activation(fl(dt_), bpxF, ACT.Abs, bias=pconst[:, 30:31], scale=1.0)
                nc.scalar.activation(fl(s1), bpxF, ACT.Sign, bias=pconst[:, 30:31], scale=1.0)
                nc.scalar.activation(fl(s1), fl(s1), ACT.Relu)       # h
                nc.vector.tensor_mul(eq[:], s1[:], pow2b)
                nc.vector.tensor_reduce(base[:], eq[:], mybir.AxisListType.X, ALU.add)
                # first ambiguous bit
                nc.vector.tensor_reduce(mred[:], dt_[:], mybir.AxisListType.X, ALU.min)
                nc.vector.tensor_tensor(eq[:], dt_[:], bcm(mred), ALU.is_equal)
                nc.vector.scalar_tensor_tensor(s1[:], eq[:], -16.0, iota16b,
                                               ALU.mult, ALU.add)
                nc.vector.tensor_reduce(k1t[:], s1[:], mybir.AxisListType.X, ALU.min)
                nc.vector.tensor_tensor(eq[:], iotab, bcm(k1t), ALU.is_equal)
                nc.vector.tensor_mul(s1[:], eq[:], pow2b)
                nc.vector.tensor_reduce(p2a[:], s1[:], mybir.AxisListType.X, ALU.add)
                nc.vector.tensor_mul(s1[:], eq[:], bpx3)
                nc.vector.tensor_reduce(bat[:], s1[:], mybir.AxisListType.X, ALU.add)
                # second ambiguous bit
                nc.vector.scalar_tensor_tensor(dm[:], eq[:], 8.0, dt_[:],
                                               ALU.mult, ALU.add)
                nc.vector.tensor_reduce(mred[:], dm[:], mybir.AxisListType.X, ALU.min)
                nc.vector.tensor_tensor(eq[:], dm[:], bcm(mred), ALU.is_equal)
                nc.vector.scalar_tensor_tensor(s1[:], eq[:], -16.0, iota16b,
                                               ALU.mult, ALU.add)
                nc.vector.tensor_reduce(k2t[:], s1[:], mybir.AxisListType.X, ALU.min)
                nc.vector.tensor_tensor(eq[:], iotab, bcm(k2t), ALU.is_equal)
                nc.vector.tensor_mul(s1[:], eq[:], pow2b)
                nc.vector.tensor_reduce(p2b[:], s1[:], mybir.AxisListType.X, ALU.add)
                nc.vector.tensor_mul(s1[:], eq[:], bpx3)
                nc.vector.tensor_reduce(bbt[:], s1[:], mybir.AxisListType.X, ALU.add)

                # ---- stage C: pair/word/weight math ([128, NT*M]) ----
                def mt(tag):
                    return msc.tile([128, NT * M], F32, tag=tag, name=tag)

                klo, khi = mt("klo"), mt("khi")
                p2lo, p2hi = mt("p2lo"), mt("p2hi")
                blo, bhi = mt("blo"), mt("bhi")
                sc1, sc2 = mt("sc1"), mt("sc2")
                nc.vector.tensor_tensor(klo[:], k1t[:], k2t[:], ALU.min)
                nc.vector.tensor_tensor(khi[:], k1t[:], k2t[:], ALU.max)
                nc.vector.tensor_tensor(p2lo[:], p2a[:], p2b[:], ALU.min)
                nc.vector.tensor_tensor(p2hi[:], p2a[:], p2b[:], ALU.max)
                nc.vector.tensor_tensor(sc1[:], k1t[:], k2t[:], ALU.is_gt)   # swap
                nc.vector.tensor_tensor(sc2[:], bbt[:], bat[:], ALU.subtract)
                nc.vector.tensor_mul(sc1[:], sc1[:], sc2[:])
                nc.vector.tensor_tensor(blo[:], bat[:], sc1[:], ALU.add)
                nc.vector.tensor_tensor(sc2[:], bat[:], bbt[:], ALU.add)
                nc.vector.tensor_tensor(bhi[:], sc2[:], blo[:], ALU.subtract)
                # pid = khi*(khi-1)/2 + klo ; gidx = pid*256 + base8
                nc.vector.scalar_tensor_tensor(sc1[:], khi[:], -1.0, khi[:],
                                               ALU.add, ALU.mult)
                nc.vector.scalar_tensor_tensor(sc1[:], sc1[:], 0.5, klo[:],
                                               ALU.mult, ALU.add)            # pid
                # base_clear
                nc.vector.tensor_scalar(sc2[:], blo[:], 0.5, None, ALU.is_gt)
                nc.vector.tensor_mul(sc2[:], sc2[:], p2lo[:])
                nc.vector.tensor_tensor(base[:], base[:], sc2[:], ALU.subtract)
                nc.vector.tensor_scalar(sc2[:], bhi[:], 0.5, None, ALU.is_gt)
                nc.vector.tensor_mul(sc2[:], sc2[:], p2hi[:])
                nc.vector.tensor_tensor(base[:], base[:], sc2[:], ALU.subtract)
                # base8 = pack(base; p2lo, p2hi) -- int domain, mod 2^k = AND(2^k-1)

                def mti(tag):
                    return msc.tile([128, NT * M], I32, tag=tag, name=tag)

                bci, loi, hii = mti("bci"), mti("loi"), mti("hii")
                t1i, t2i = mti("t1i"), mti("t2i")
                lowv = mt("lowv")
                nc.vector.tensor_copy(bci[:], base[:])
                nc.vector.tensor_copy(loi[:], p2lo[:])
                nc.vector.tensor_copy(hii[:], p2hi[:])
                nc.vector.tensor_scalar(loi[:], loi[:], -1, None, ALU.add)
                nc.vector.tensor_scalar(hii[:], hii[:], -1, None, ALU.add)
                nc.vector.tensor_tensor(loi[:], bci[:], loi[:], ALU.bitwise_and)
                nc.vector.tensor_tensor(hii[:], bci[:], hii[:], ALU.bitwise_and)
                nc.vector.tensor_tensor(t1i[:], hii[:], loi[:], ALU.subtract)
                nc.vector.tensor_scalar(t1i[:], t1i[:], 1, None, ALU.arith_shift_right)
                nc.vector.tensor_tensor(t2i[:], bci[:], hii[:], ALU.subtract)
                nc.vector.tensor_scalar(t2i[:], t2i[:], 2, None, ALU.arith_shift_right)
                nc.vector.tensor_tensor(t1i[:], t1i[:], t2i[:], ALU.add)
                nc.vector.tensor_tensor(t1i[:], t1i[:], loi[:], ALU.add)     # base8 int
                nc.vector.tensor_copy(lowv[:], t1i[:])                       # base8 f32
                nc.vector.scalar_tensor_tensor(lowv[:], sc1[:], 256.0, lowv[:],
                                               ALU.mult, ALU.add)            # gidx
                # weights, m-major: [128, M, NT, 4]
                wt = wp.tile([128, M, NT, 4], F32, tag="wt")
                nc.vector.tensor_scalar(sc1[:], blo[:], -1.0, 1.0, ALU.mult, ALU.add)
                nc.vector.tensor_scalar(sc2[:], bhi[:], -1.0, 1.0, ALU.mult, ALU.add)

                def wslot(jj):  # [128, M, NT] view ordered as (t, m)
                    return wt[:, :, :, jj].rearrange("p m t -> p t m")

                def v3(t):  # [128,TM] -> [128, NT, M]
                    return t[:].rearrange("p (t m) -> p t m", t=NT, m=M)

                nc.vector.tensor_mul(wslot(0), v3(sc1), v3(sc2))
                nc.vector.tensor_mul(wslot(1), v3(blo), v3(sc2))
                nc.vector.tensor_mul(wslot(2), v3(sc1), v3(bhi))
                nc.vector.tensor_mul(wslot(3), v3(blo), v3(bhi))

                # ---- stage D: PE fold of idxs to wrapped [16, n/16] layout ----
                idxt = idxp.tile([128, M, NT, 8], I16, tag="idxt")
                for f in range(8):
                    fold = dps.tile([16, NT * M], F32, tag="fold")
                    nc.tensor.matmul(fold[:], ident[:, 16 * f:16 * (f + 1)],
                                     lowv[:], start=True, stop=True)
                    nc.scalar.copy(
                        idxt[0:16, :, :, f],
                        fold[:].rearrange("q (t m) -> q m t", t=NT, m=M))
                for g in range(1, 8):
                    nc.scalar.dma_start(idxt[16 * g:16 * (g + 1), :, :, :],
                                        idxt[0:16, :, :, :])

                # ---- stage E: gather + weighted vote reduce ----
                for m in range(M):
                    v = vp.tile([128, NT, 4, D], F32, tag="v")
                    if os.environ.get("KBISECT") == "nogather":
                        nc.vector.memset(v[:].rearrange("p t j d -> p (t j d)"), 0)
                    else:
                        nc.gpsimd.dma_gather(
                            out_ap=v[:].rearrange("p t j d -> p t (j d)"),
                            in_ap=pt_p.ap()[m],
                            idxs_ap=idxt[:, m, :, :],
                            num_idxs=NPX,
                            num_idxs_reg=NPX,
                            elem_size=4 * D,
                            single_packet=False,
                            queue_num=m % 4,
                        )
                    wv = wvp.tile([128, NT, 4, D], F32, tag="wv")
                    wb = wt[:, m, :, :].rearrange("p t j -> p (t j)") \
                        .unsqueeze(-1).broadcast_to([128, NT * 4, D])
                    nc.vector.tensor_tensor(
                        wv[:].rearrange("p t j d -> p (t j) d"),
                        v[:].rearrange("p t j d -> p (t j) d"), wb, ALU.mult)
                    nc.vector.tensor_add(
                        wv[:, :, 0:2, :].rearrange("p t j d -> p t (j d)"),
                        wv[:, :, 0:2, :].rearrange("p t j d -> p t (j d)"),
                        wv[:, :, 2:4, :].rearrange("p t j d -> p t (j d)"))
                    nc.vector.tensor_add(wv[:, :, 0, :], wv[:, :, 0, :],
                                         wv[:, :, 1, :])
                    if m == 0:
                        nc.scalar.copy(feat[:, img], wv[:, :, 0, :])
                    else:
                        nc.vector.tensor_add(feat[:, img], feat[:, img],
                                             wv[:, :, 0, :])

                if img + 1 < NI:
                    emit_stage_a(img + 1)

                # ---- stage F: 2x2 avg pool via PE ----
                pps_t = pps.tile([128, 8, D], F32, tag="pp")
                for g in range(8):
                    for s in range(4):
                        nc.tensor.matmul(
                            pps_t[:, g, :],
                            poolw[:, s * 128:(s + 1) * 128],
                            feat[:, img, 4 * g + s, :],
                            start=(s == 0), stop=(s == 3))
                nc.scalar.copy(
                    flatbuf[:, :, :, img],
                    pps_t[:].rearrange("p g d -> p d g"))

            # ---- classifier ----
            lg = lps.tile([NCLS, NI], F32)
            wqv = wqt[:].rearrange("p (c l) -> p c l", c=D * 8, l=NCLS)
            flv = flatbuf[:].rearrange("p d g i -> p (d g) i")
            for c_ in range(D * 8):
                nc.tensor.matmul(lg[:], wqv[:, c_, :], flv[:, c_, :],
                                 start=(c_ == 0), stop=(c_ == D * 8 - 1))
            lsb = flp.tile([NCLS, NI], F32)
            nc.scalar.activation(lsb[:], lg[:], ACT.Identity, bias=bpred[:], scale=1.0)
            nc.sync.dma_start(out_p.ap(), lsb[:])

    nc.compile()
    return nc


_CACHE: dict = {}


def _get_kernel(c1, c2, dy1, dx1, dy2, dx2):
    key = (c1.tobytes(), c2.tobytes(), dy1.tobytes(), dx1.tobytes(),
           dy2.tobytes(), dx2.tobytes())
    if key not in _CACHE:
        _CACHE[key] = _build_kernel(c1, c2, dy1, dx1, dy2, dx2)
    return _CACHE[key]


def kernel(x, c1, c2, dy1, dx1, dy2, dx2, thresholds, table, w_pred, b_pred):
    x = np.asarray(x, dtype=np.float32)
    c1, c2 = np.asarray(c1, np.int32), np.asarray(c2, np.int32)
    dy1, dx1 = np.asarray(dy1, np.int32), np.asarray(dx1, np.int32)
    dy2, dx2 = np.asarray(dy2, np.int32), np.asarray(dx2, np.int32)
    thresholds = np.asarray(thresholds, np.float32)
    table = np.asarray(table, np.float32)
    w_pred = np.asarray(w_pred, np.float32)
    b_pred = np.asarray(b_pred, np.float32)

    nc = _get_kernel(c1, c2, dy1, dx1, dy2, dx2)

    xp = np.pad(x, ((0, 0), (0, 0), (0, L - 1), (0, L - 1)))
    PT = _build_pair_table(table)
    pconst, pwsel, ones4k, ident, poolW, wqT, bpred = _host_consts(
        thresholds, w_pred, b_pred, c1, c2, dy1, dx1, dy2, dx2)

    in_maps = []
    for c in range(NCORES):
        in_maps.append(dict(
            xp=np.ascontiguousarray(xp[c * NI:(c + 1) * NI]),
            pt=PT, pconst=pconst, pwsel=pwsel, ones4k=ones4k, ident=ident,
            poolw=poolW, wqt=wqT, bpred=bpred,
        ))
    res = run_bass_kernel_spmd(nc, in_maps, core_ids=list(range(NCORES)))
    outs = [r["out"].T for r in res.results]      # each [NI, NCLS]
    return np.concatenate(outs, axis=0).astype(np.float32)


# revision 28
# speedup vs baseline: 1.9004x; 1.0687x over previous
"""Trainium2 Bass kernel for the fern/sparse-table CTE model.

Strategy: data-parallel over batch N=32 across 8 cores (4 images each).
Stage A loads all 108 possible (c,dy,dx) windows with one overlapping-window
DMA per plane, then a PE matmul against a +1/-1 selection matrix (with the
threshold folded in via an accumulated ones-row matmul) produces thresholded
pixel-pair differences directly in transposed (pixel-major) layout; sigmoid
on the Scalar engine gives soft bits. Per-site top-2 ambiguous-bit extraction
runs on DVE; the T=4 table-row gather is served by dma_gather from a
host-prebuilt "pair table" whose 256B elements hold the 4 rows for a packed
(pair-id, base8) index. Gathers for the 8 ferns round-robin over the 4 SWDGE
queues so up to 4 descriptor-generation kernels run concurrently on distinct
Q7 core pairs. Votes are weighted and reduced on DVE; 2x2 average pooling and
the classifier run as PE matmuls.
"""
import os
import numpy as np
from contextlib import ExitStack

import concourse.bacc as bacc
import concourse.hw_specs as _hw_specs

# The stock cost model prices SWDGE descriptor generation at 0.34 ns/desc;
# dma_gather's Q7 kernel measures ~8 ns/idx on TRN2 hardware. The tile
# scheduler orders engine programs from this model, so feed it the measured
# rate or it schedules the gather pipeline hopelessly optimistically.
_hw_specs.TRN2Spec.SWDGE_NS_PER_DESCRIPTOR = 7.8
import concourse.bass as bass
import concourse.tile as tile
from concourse import mybir
from concourse.bass_utils import run_bass_kernel_spmd

F32 = mybir.dt.float32
BF16 = mybir.dt.bfloat16
FP16 = mybir.dt.float16
I16 = mybir.dt.int16
I32 = mybir.dt.int32
ALU = mybir.AluOpType
ACT = mybir.ActivationFunctionType

M, K, L = 8, 10, 6
D = 16                      # D_OUT
NCLS = 10
N, C, H, W = 32, 3, 64, 64
NCORES = 8
NI = N // NCORES            # images per core
HP = H + L - 1              # 69 padded
NPX = H * W                 # 4096
NT = NPX // 128             # 32 pixel tiles per image
NPAIR = 45
PTROWS = NPAIR * 256        # 11520 elements per fern
NWIN = C * L * L            # 108 distinct windows


def _build_pair_table(table: np.ndarray) -> np.ndarray:
    """PT[m, pid*256+base8, 64] f32; rows j=ilo+2*ihi of the 256B element are
    table[m*1024 + unpack(base8;klo,khi) + ilo*2^klo + ihi*2^khi]."""
    tbl = table.reshape(M, 1024, D)
    PT = np.zeros((M, PTROWS, 4 * D), dtype=np.float32)
    base8 = np.arange(256)
    for khi in range(K):
        for klo in range(khi):
            pid = khi * (khi - 1) // 2 + klo
            rest = [k for k in range(K) if k not in (klo, khi)]
            unpacked = np.zeros(256, dtype=np.int64)
            for r, k in enumerate(rest):
                unpacked += ((base8 >> r) & 1) << k
            for ihi in range(2):
                for ilo in range(2):
                    j = ilo + 2 * ihi
                    rows = unpacked + ilo * (1 << klo) + ihi * (1 << khi)
                    PT[:, pid * 256 + base8, j * D:(j + 1) * D] = tbl[:, rows, :]
    return PT


def _host_consts(thresholds, w_pred, b_pred, c1, c2, dy1, dx1, dy2, dx2):
    pconst = np.zeros((128, 32), dtype=np.float32)
    pconst[:, 30] = -0.5
    pconst[:, 0:10] = (1 << np.arange(K)).astype(np.float32)[None, :]
    pconst[:, 10:20] = np.arange(K, dtype=np.float32)[None, :]
    pconst[:, 20:30] = np.arange(K, dtype=np.float32)[None, :] + 16.0
    # window-selection matrix with thresholds in row 108 (paired with a ones
    # lhsT row): z[site, r] = win1_r[site] - win2_r[site] - thr_r
    pw = np.zeros((128, M * K), dtype=np.float32)
    w1 = (np.asarray(c1) * 36 + np.asarray(dy1) * 6 + np.asarray(dx1)).reshape(-1)
    w2 = (np.asarray(c2) * 36 + np.asarray(dy2) * 6 + np.asarray(dx2)).reshape(-1)
    for r in range(M * K):
        pw[w1[r], r] += 1.0
        pw[w2[r], r] -= 1.0
    pw[NWIN, :] = -thresholds.reshape(-1)
    ones4k = np.ones((1, NPX), dtype=np.float32)
    ident = np.eye(128, dtype=np.float32)
    # pool lhsT: poolW[p, s*128 + s2*32 + w2] = 0.25 if s2==s and (p%64)//2==w2
    poolW = np.zeros((128, 4, 4, 32), dtype=np.float32)
    p = np.arange(128)
    for s in range(4):
        poolW[p, s, s, (p % 64) // 2] = 0.25
    poolW = poolW.reshape(128, 512)
    # classifier lhsT: wqT[p, c, cls] = w_pred[cls, d*1024 + (4g+s)*32 + w2]
    # with c = d*8+g, p = s*32+w2
    wq = w_pred.reshape(NCLS, D, 8, 4, 32)          # [cls, d, g, s, w2]
    wqT = np.transpose(wq, (3, 4, 1, 2, 0)).reshape(128, D * 8, NCLS)
    wqT = np.ascontiguousarray(wqT.reshape(128, D * 8 * NCLS)).astype(np.float32)
    bpred = b_pred.reshape(NCLS, 1).astype(np.float32)
    return pconst, pw, ones4k, ident, poolW, wqT, bpred


def _build_kernel(c1, c2, dy1, dx1, dy2, dx2):
    """Build + compile the per-core kernel. Fern geometry is baked into the
    selection matrix at trace time."""
    nc = bacc.Bacc("TRN2", num_devices=NCORES, num_swdge_queues=4)

    xp_p = nc.declare_dram_parameter("xp", [NI, C, HP, HP], FP16, isOutput=False)
    pt_p = nc.declare_dram_parameter("pt", [M, PTROWS, 4 * D], F32, isOutput=False)
    pc_p = nc.declare_dram_parameter("pconst", [128, 32], F32, isOutput=False)
    pw_p = nc.declare_dram_parameter("pwsel", [128, M * K], FP16, isOutput=False)
    on_p = nc.declare_dram_parameter("ones4k", [1, NPX], FP16, isOutput=False)
    id_p = nc.declare_dram_parameter("ident", [128, 128], F32, isOutput=False)
    pl_p = nc.declare_dram_parameter("poolw", [128, 512], F32, isOutput=False)
    wq_p = nc.declare_dram_parameter("wqt", [128, D * 8 * NCLS], F32, isOutput=False)
    bp_p = nc.declare_dram_parameter("bpred", [NCLS, 1], F32, isOutput=False)
    out_p = nc.declare_dram_parameter("out", [NCLS, NI], F32, isOutput=True)

    with tile.TileContext(nc, num_cores=NCORES) as tc:
        with ExitStack() as ctx:
            cpool = ctx.enter_context(tc.tile_pool(name="consts", bufs=1))
            awp = ctx.enter_context(tc.tile_pool(name="allwin", bufs=2))
            bpxp = ctx.enter_context(tc.tile_pool(name="bpx", bufs=2))
            kp = ctx.enter_context(tc.tile_pool(name="kstage", bufs=1))
            mp = ctx.enter_context(tc.tile_pool(name="mstage", bufs=1))
            msc = ctx.enter_context(tc.tile_pool(name="mscratch", bufs=1))
            wp = ctx.enter_context(tc.tile_pool(name="wgidx", bufs=2))
            idxp = ctx.enter_context(tc.tile_pool(name="idx", bufs=2))
            vp = ctx.enter_context(tc.tile_pool(name="votes", bufs=8))
            wvp = ctx.enter_context(tc.tile_pool(name="wv", bufs=1))
            fp = ctx.enter_context(tc.tile_pool(name="feat", bufs=1))
            flp = ctx.enter_context(tc.tile_pool(name="flat", bufs=1))
            tps = ctx.enter_context(tc.tile_pool(name="tpsum", bufs=3, space="PSUM"))
            dps = ctx.enter_context(tc.tile_pool(name="dpsum", bufs=2, space="PSUM"))
            pps = ctx.enter_context(tc.tile_pool(name="ppsum", bufs=2, space="PSUM"))
            lps = ctx.enter_context(tc.tile_pool(name="lpsum", bufs=1, space="PSUM"))

            # ---- constants ----
            pconst = cpool.tile([128, 32], F32)
            nc.sync.dma_start(pconst[:], pc_p.ap())
            pwsel = cpool.tile([128, M * K], FP16)
            nc.sync.dma_start(pwsel[:], pw_p.ap())
            ones4k = cpool.tile([1, NPX], F32)
            nc.sync.dma_start(ones4k[:], on_p.ap())
            ident = cpool.tile([128, 128], F32)
            nc.sync.dma_start(ident[:], id_p.ap())
            poolw = cpool.tile([128, 512], F32)
            nc.sync.dma_start(poolw[:], pl_p.ap())
            wqt = cpool.tile([128, D * 8 * NCLS], F32)
            nc.sync.dma_start(wqt[:], wq_p.ap())
            bpred = cpool.tile([NCLS, 1], F32)
            nc.sync.dma_start(bpred[:], bp_p.ap())

            def bc10(col):  # [128,10] const col -> [128,NT*M,10] broadcast
                v = pconst[:, col:col + 10]
                return v.unsqueeze(1).broadcast_to([128, NT * M, K])

            pow2b, iotab, iota16b = bc10(0), bc10(10), bc10(20)

            feat = fp.tile([128, NI, NT, D], F32)
            flatbuf = flp.tile([128, D, 8, NI], F32)

            bpx_tiles = [None] * NI

            def emit_stage_a(img):
                # ---- stage A: all-windows load + select/transpose matmul ----
                allwin = awp.tile([128, NPX], FP16, tag="allwin")
                for c in range(C):
                    for dy in range(L):
                        src = bass.AP(xp_p.ap().tensor,
                                      (img * C + c) * HP * HP + dy * HP,
                                      [[1, L], [HP, H], [1, W]])
                        w0 = c * 36 + dy * 6
                        nc.sync.dma_start(
                            allwin[w0:w0 + 6, :]
                            .rearrange("p (a b) -> p a b", a=H, b=W), src)
                nc.sync.dma_start(allwin[NWIN:NWIN + 1, :], on_p.ap())

                bpx = bpxp.tile([128, NT, M, K], F32, tag="bpx")
                bpx_tiles[img] = bpx
                done = 0
                while done < NT:
                    grp = min(4, NT - done)
                    tp = tps.tile([128, 4 * M * K], F32, tag="tp")
                    for i in range(grp):
                        t_ = done + i
                        nc.tensor.matmul(
                            tp[:, i * 80:(i + 1) * 80],
                            allwin[0:NWIN + 1, t_ * 128:(t_ + 1) * 128],
                            pwsel[0:NWIN + 1, :], start=True, stop=True)
                    nc.scalar.activation(
                        bpx[:, done:done + grp, :, :]
                        .rearrange("p t m k -> p (t m k)"),
                        tp[:, 0:80 * grp], ACT.Sigmoid)
                    done += grp

            emit_stage_a(0)
            for img in range(NI):
                bpx = bpx_tiles[img]
                # ---- stage B: per-site bit stats (pixel layout) ----
                TM = NT * M
                dt_ = kp.tile([128, TM, K], F32, tag="dt")
                eq = kp.tile([128, TM, K], F32, tag="eq")
                s1 = kp.tile([128, TM, K], F32, tag="s1")
                dm = dt_

                base = mp.tile([128, TM], F32, tag="base")
                k1t = mp.tile([128, TM], F32, tag="k1t")
                p2a = mp.tile([128, TM], F32, tag="p2a")
                bat = mp.tile([128, TM], F32, tag="bat")
                k2t = mp.tile([128, TM], F32, tag="k2t")
                p2b = mp.tile([128, TM], F32, tag="p2b")
                bbt = mp.tile([128, TM], F32, tag="bbt")
                mred = mp.tile([128, TM], F32, tag="mred")

                bpx3 = bpx[:].rearrange("p t m k -> p (t m) k")
                bpxF = bpx[:].rearrange("p t m k -> p (t m k)")

                def fl(t):  # [128,TM,K] -> flat 2D
                    return t[:].rearrange("p s k -> p (s k)")

                def bcm(t):  # [128,TM] -> broadcast over K
                    return t[:].unsqueeze(-1).broadcast_to([128, TM, K])

                nc.scalar.activation(fl(dt_), bpxF, ACT.Abs, bias=pconst[:, 30:31], scale=1.0)
                nc.scalar.activation(fl(s1), bpxF, ACT.Sign, bias=pconst[:, 30:31], scale=1.0)
                nc.scalar.activation(fl(s1), fl(s1), ACT.Relu)       # h
                nc.vector.tensor_mul(eq[:], s1[:], pow2b)
                nc.vector.tensor_reduce(base[:], eq[:], mybir.AxisListType.X, ALU.add)
                # first ambiguous bit
                nc.vector.tensor_reduce(mred[:], dt_[:], mybir.AxisListType.X, ALU.min)
                nc.vector.tensor_tensor(eq[:], dt_[:], bcm(mred), ALU.is_equal)
                nc.vector.scalar_tensor_tensor(s1[:], eq[:], -16.0, iota16b,
                                               ALU.mult, ALU.add)
                nc.vector.tensor_reduce(k1t[:], s1[:], mybir.AxisListType.X, ALU.min)
                nc.vector.tensor_tensor(eq[:], iotab, bcm(k1t), ALU.is_equal)
                nc.vector.tensor_mul(s1[:], eq[:], pow2b)
                nc.vector.tensor_reduce(p2a[:], s1[:], mybir.AxisListType.X, ALU.add)
                nc.vector.tensor_mul(s1[:], eq[:], bpx3)
                nc.vector.tensor_reduce(bat[:], s1[:], mybir.AxisListType.X, ALU.add)
                # second ambiguous bit
                nc.vector.scalar_tensor_tensor(dm[:], eq[:], 8.0, dt_[:],
                                               ALU.mult, ALU.add)
                nc.vector.tensor_reduce(mred[:], dm[:], mybir.AxisListType.X, ALU.min)
                nc.vector.tensor_tensor(eq[:], dm[:], bcm(mred), ALU.is_equal)
                nc.vector.scalar_tensor_tensor(s1[:], eq[:], -16.0, iota16b,
                                               ALU.mult, ALU.add)
                nc.vector.tensor_reduce(k2t[:], s1[:], mybir.AxisListType.X, ALU.min)
                nc.vector.tensor_tensor(eq[:], iotab, bcm(k2t), ALU.is_equal)
                nc.vector.tensor_mul(s1[:], eq[:], pow2b)
                nc.vector.tensor_reduce(p2b[:], s1[:], mybir.AxisListType.X, ALU.add)
                nc.vector.tensor_mul(s1[:], eq[:], bpx3)
                nc.vector.tensor_reduce(bbt[:], s1[:], mybir.AxisListType.X, ALU.add)

                # ---- stage C: pair/word/weight math ([128, NT*M]) ----
                def mt(tag):
                    return msc.tile([128, NT * M], F32, tag=tag, name=tag)

                klo, khi = mt("klo"), mt("khi")
                p2lo, p2hi = mt("p2lo"), mt("p2hi")
                blo, bhi = mt("blo"), mt("bhi")
                sc1, sc2 = mt("sc1"), mt("sc2")
                nc.vector.tensor_tensor(klo[:], k1t[:], k2t[:], ALU.min)
                nc.vector.tensor_tensor(khi[:], k1t[:], k2t[:], ALU.max)
                nc.vector.tensor_tensor(p2lo[:], p2a[:], p2b[:], ALU.min)
                nc.vector.tensor_tensor(p2hi[:], p2a[:], p2b[:], ALU.max)
                nc.vector.tensor_tensor(sc1[:], k1t[:], k2t[:], ALU.is_gt)   # swap
                nc.vector.tensor_tensor(sc2[:], bbt[:], bat[:], ALU.subtract)
                nc.vector.tensor_mul(sc1[:], sc1[:], sc2[:])
                nc.vector.tensor_tensor(blo[:], bat[:], sc1[:], ALU.add)
                nc.vector.tensor_tensor(sc2[:], bat[:], bbt[:], ALU.add)
                nc.vector.tensor_tensor(bhi[:], sc2[:], blo[:], ALU.subtract)
                # pid = khi*(khi-1)/2 + klo ; gidx = pid*256 + base8
                nc.vector.scalar_tensor_tensor(sc1[:], khi[:], -1.0, khi[:],
                                               ALU.add, ALU.mult)
                nc.vector.scalar_tensor_tensor(sc1[:], sc1[:], 0.5, klo[:],
                                               ALU.mult, ALU.add)            # pid
                # base_clear
                nc.vector.tensor_scalar(sc2[:], blo[:], 0.5, None, ALU.is_gt)
                nc.vector.tensor_mul(sc2[:], sc2[:], p2lo[:])
                nc.vector.tensor_tensor(base[:], base[:], sc2[:], ALU.subtract)
                nc.vector.tensor_scalar(sc2[:], bhi[:], 0.5, None, ALU.is_gt)
                nc.vector.tensor_mul(sc2[:], sc2[:], p2hi[:])
                nc.vector.tensor_tensor(base[:], base[:], sc2[:], ALU.subtract)
                # base8 = pack(base; p2lo, p2hi) -- int domain, mod 2^k = AND(2^k-1)

                def mti(tag):
                    return msc.tile([128, NT * M], I32, tag=tag, name=tag)

                bci, loi, hii = mti("bci"), mti("loi"), mti("hii")
                t1i, t2i = mti("t1i"), mti("t2i")
                lowv = mt("lowv")
                nc.vector.tensor_copy(bci[:], base[:])
                nc.vector.tensor_copy(loi[:], p2lo[:])
                nc.vector.tensor_copy(hii[:], p2hi[:])
                nc.vector.tensor_scalar(loi[:], loi[:], -1, None, ALU.add)
                nc.vector.tensor_scalar(hii[:], hii[:], -1, None, ALU.add)
                nc.vector.tensor_tensor(loi[:], bci[:], loi[:], ALU.bitwise_and)
                nc.vector.tensor_tensor(hii[:], bci[:], hii[:], ALU.bitwise_and)
                nc.vector.tensor_tensor(t1i[:], hii[:], loi[:], ALU.subtract)
                nc.vector.tensor_scalar(t1i[:], t1i[:], 1, None, ALU.arith_shift_right)
                nc.vector.tensor_tensor(t2i[:], bci[:], hii[:], ALU.subtract)
                nc.vector.tensor_scalar(t2i[:], t2i[:], 2, None, ALU.arith_shift_right)
                nc.vector.tensor_tensor(t1i[:], t1i[:], t2i[:], ALU.add)
                nc.vector.tensor_tensor(t1i[:], t1i[:], loi[:], ALU.add)     # base8 int
                nc.vector.tensor_copy(lowv[:], t1i[:])                       # base8 f32
                nc.vector.scalar_tensor_tensor(lowv[:], sc1[:], 256.0, lowv[:],
                                               ALU.mult, ALU.add)            # gidx
                # weights, m-major: [128, M, NT, 4]
                wt = wp.tile([128, M, NT, 4], F32, tag="wt")
                nc.vector.tensor_scalar(sc1[:], blo[:], -1.0, 1.0, ALU.mult, ALU.add)
                nc.vector.tensor_scalar(sc2[:], bhi[:], -1.0, 1.0, ALU.mult, ALU.add)

                def wslot(jj):  # [128, M, NT] view ordered as (t, m)
                    return wt[:, :, :, jj].rearrange("p m t -> p t m")

                def v3(t):  # [128,TM] -> [128, NT, M]
                    return t[:].rearrange("p (t m) -> p t m", t=NT, m=M)

                nc.vector.tensor_mul(wslot(0), v3(sc1), v3(sc2))
                nc.vector.tensor_mul(wslot(1), v3(blo), v3(sc2))
                nc.vector.tensor_mul(wslot(2), v3(sc1), v3(bhi))
                nc.vector.tensor_mul(wslot(3), v3(blo), v3(bhi))

                # ---- stage D: PE fold of idxs to wrapped [16, n/16] layout ----
                idxt = idxp.tile([128, M, NT, 8], I16, tag="idxt")
                for f in range(8):
                    fold = dps.tile([16, NT * M], F32, tag="fold")
                    nc.tensor.matmul(fold[:], ident[:, 16 * f:16 * (f + 1)],
                                     lowv[:], start=True, stop=True)
                    nc.scalar.copy(
                        idxt[0:16, :, :, f],
                        fold[:].rearrange("q (t m) -> q m t", t=NT, m=M))
                for g in range(1, 8):
                    nc.scalar.dma_start(idxt[16 * g:16 * (g + 1), :, :, :],
                                        idxt[0:16, :, :, :])

                # ---- stage E: gather + weighted vote reduce ----
                for m in range(M):
                    v = vp.tile([128, NT, 4, D], F32, tag="v")
                    if os.environ.get("KBISECT") == "nogather":
                        nc.vector.memset(v[:].rearrange("p t j d -> p (t j d)"), 0)
                    else:
                        nc.gpsimd.dma_gather(
                            out_ap=v[:].rearrange("p t j d -> p t (j d)"),
                            in_ap=pt_p.ap()[m],
                            idxs_ap=idxt[:, m, :, :],
                            num_idxs=NPX,
                            num_idxs_reg=NPX,
                            elem_size=4 * D,
                            single_packet=False,
                            queue_num=m % 4,
                        )
                    wv = wvp.tile([128, NT, 4, D], F32, tag="wv")
                    wb = wt[:, m, :, :].rearrange("p t j -> p (t j)") \
                        .unsqueeze(-1).broadcast_to([128, NT * 4, D])
                    nc.vector.tensor_tensor(
                        wv[:].rearrange("p t j d -> p (t j) d"),
                        v[:].rearrange("p t j d -> p (t j) d"), wb, ALU.mult)
                    nc.vector.tensor_add(
                        wv[:, :, 0:2, :].rearrange("p t j d -> p t (j d)"),
                        wv[:, :, 0:2, :].rearrange("p t j d -> p t (j d)"),
                        wv[:, :, 2:4, :].rearrange("p t j d -> p t (j d)"))
                    nc.vector.tensor_add(wv[:, :, 0, :], wv[:, :, 0, :],
                                         wv[:, :, 1, :])
                    if m == 0:
                        nc.scalar.copy(feat[:, img], wv[:, :, 0, :])
                    else:
                        nc.vector.tensor_add(feat[:, img], feat[:, img],
                                             wv[:, :, 0, :])

                if img + 1 < NI:
                    emit_stage_a(img + 1)

                # ---- stage F: 2x2 avg pool via PE ----
                pps_t = pps.tile([128, 8, D], F32, tag="pp")
                for g in range(8):
                    for s in range(4):
                        nc.tensor.matmul(
                            pps_t[:, g, :],
                            poolw[:, s * 128:(s + 1) * 128],
                            feat[:, img, 4 * g + s, :],
                            start=(s == 0), stop=(s == 3))
                nc.scalar.copy(
                    flatbuf[:, :, :, img],
                    pps_t[:].rearrange("p g d -> p d g"))

            # ---- classifier ----
            lg = lps.tile([NCLS, NI], F32)
            wqv = wqt[:].rearrange("p (c l) -> p c l", c=D * 8, l=NCLS)
            flv = flatbuf[:].rearrange("p d g i -> p (d g) i")
            for c_ in range(D * 8):
                nc.tensor.matmul(lg[:], wqv[:, c_, :], flv[:, c_, :],
                                 start=(c_ == 0), stop=(c_ == D * 8 - 1))
            lsb = flp.tile([NCLS, NI], F32)
            nc.scalar.activation(lsb[:], lg[:], ACT.Identity, bias=bpred[:], scale=1.0)
            nc.sync.dma_start(out_p.ap(), lsb[:])

    nc.compile()
    return nc


_CACHE: dict = {}


def _get_kernel(c1, c2, dy1, dx1, dy2, dx2):
    key = (c1.tobytes(), c2.tobytes(), dy1.tobytes(), dx1.tobytes(),
           dy2.tobytes(), dx2.tobytes())
    if key not in _CACHE:
        _CACHE[key] = _build_kernel(c1, c2, dy1, dx1, dy2, dx2)
    return _CACHE[key]


def kernel(x, c1, c2, dy1, dx1, dy2, dx2, thresholds, table, w_pred, b_pred):
    x = np.asarray(x, dtype=np.float32)
    c1, c2 = np.asarray(c1, np.int32), np.asarray(c2, np.int32)
    dy1, dx1 = np.asarray(dy1, np.int32), np.asarray(dx1, np.int32)
    dy2, dx2 = np.asarray(dy2, np.int32), np.asarray(dx2, np.int32)
    thresholds = np.asarray(thresholds, np.float32)
    table = np.asarray(table, np.float32)
    w_pred = np.asarray(w_pred, np.float32)
    b_pred = np.asarray(b_pred, np.float32)

    nc = _get_kernel(c1, c2, dy1, dx1, dy2, dx2)

    xp = np.pad(x, ((0, 0), (0, 0), (0, L - 1), (0, L - 1)))
    PT = _build_pair_table(table)
    pconst, pwsel, ones4k, ident, poolW, wqT, bpred = _host_consts(
        thresholds, w_pred, b_pred, c1, c2, dy1, dx1, dy2, dx2)

    in_maps = []
    for c in range(NCORES):
        in_maps.append(dict(
            xp=np.ascontiguousarray(xp[c * NI:(c + 1) * NI]),
            pt=PT, pconst=pconst, pwsel=pwsel, ones4k=ones4k, ident=ident,
            poolw=poolW, wqt=wqT, bpred=bpred,
        ))
    res = run_bass_kernel_spmd(nc, in_maps, core_ids=list(range(NCORES)))
    outs = [r["out"].T for r in res.results]      # each [NI, NCLS]
    return np.concatenate(outs, axis=0).astype(np.float32)


# revision 29
# speedup vs baseline: 1.9722x; 1.0378x over previous
"""Trainium2 Bass kernel for the fern/sparse-table CTE model.

Strategy: data-parallel over batch N=32 across 8 cores (4 images each).
Stage A loads all 108 possible (c,dy,dx) windows with one overlapping-window
DMA per plane, then a PE matmul against a +1/-1 selection matrix (with the
threshold folded in via an accumulated ones-row matmul) produces thresholded
pixel-pair differences directly in transposed (pixel-major) layout; sigmoid
on the Scalar engine gives soft bits. Per-site top-2 ambiguous-bit extraction
runs on DVE; the T=4 table-row gather is served by dma_gather from a
host-prebuilt "pair table" whose 256B elements hold the 4 rows for a packed
(pair-id, base8) index. Gathers for the 8 ferns round-robin over the 4 SWDGE
queues so up to 4 descriptor-generation kernels run concurrently on distinct
Q7 core pairs. Votes are weighted and reduced on DVE; 2x2 average pooling and
the classifier run as PE matmuls.
"""
import os
import numpy as np
from contextlib import ExitStack

import concourse.bacc as bacc
import concourse.bass as bass
import concourse.tile as tile
from concourse import mybir
from concourse.bass_utils import run_bass_kernel_spmd

F32 = mybir.dt.float32
BF16 = mybir.dt.bfloat16
FP16 = mybir.dt.float16
I16 = mybir.dt.int16
I32 = mybir.dt.int32
ALU = mybir.AluOpType
ACT = mybir.ActivationFunctionType

M, K, L = 8, 10, 6
D = 16                      # D_OUT
NCLS = 10
N, C, H, W = 32, 3, 64, 64
NCORES = 8
NI = N // NCORES            # images per core
HP = H + L - 1              # 69 padded
NPX = H * W                 # 4096
NT = NPX // 128             # 32 pixel tiles per image
NPAIR = 45
PTROWS = NPAIR * 256        # 11520 elements per fern
NWIN = C * L * L            # 108 distinct windows


def _build_pair_table(table: np.ndarray) -> np.ndarray:
    """PT[m, pid*256+base8, 64] f32; rows j=ilo+2*ihi of the 256B element are
    table[m*1024 + unpack(base8;klo,khi) + ilo*2^klo + ihi*2^khi]."""
    tbl = table.reshape(M, 1024, D)
    PT = np.zeros((M, PTROWS, 4 * D), dtype=np.float32)
    base8 = np.arange(256)
    for khi in range(K):
        for klo in range(khi):
            pid = khi * (khi - 1) // 2 + klo
            rest = [k for k in range(K) if k not in (klo, khi)]
            unpacked = np.zeros(256, dtype=np.int64)
            for r, k in enumerate(rest):
                unpacked += ((base8 >> r) & 1) << k
            for ihi in range(2):
                for ilo in range(2):
                    j = ilo + 2 * ihi
                    rows = unpacked + ilo * (1 << klo) + ihi * (1 << khi)
                    PT[:, pid * 256 + base8, j * D:(j + 1) * D] = tbl[:, rows, :]
    return PT


def _host_consts(thresholds, w_pred, b_pred, c1, c2, dy1, dx1, dy2, dx2):
    pconst = np.zeros((128, 32), dtype=np.float32)
    pconst[:, 30] = -0.5
    pconst[:, 0:10] = (1 << np.arange(K)).astype(np.float32)[None, :]
    pconst[:, 10:20] = np.arange(K, dtype=np.float32)[None, :]
    pconst[:, 20:30] = np.arange(K, dtype=np.float32)[None, :] + 16.0
    # window-selection matrix with thresholds in row 108 (paired with a ones
    # lhsT row): z[site, r] = win1_r[site] - win2_r[site] - thr_r
    pw = np.zeros((128, M * K), dtype=np.float32)
    w1 = (np.asarray(c1) * 36 + np.asarray(dy1) * 6 + np.asarray(dx1)).reshape(-1)
    w2 = (np.asarray(c2) * 36 + np.asarray(dy2) * 6 + np.asarray(dx2)).reshape(-1)
    for r in range(M * K):
        pw[w1[r], r] += 1.0
        pw[w2[r], r] -= 1.0
    pw[NWIN, :] = -thresholds.reshape(-1)
    ones4k = np.ones((1, NPX), dtype=np.float32)
    ident = np.eye(128, dtype=np.float32)
    # pool lhsT: poolW[p, s*128 + s2*32 + w2] = 0.25 if s2==s and (p%64)//2==w2
    poolW = np.zeros((128, 4, 4, 32), dtype=np.float32)
    p = np.arange(128)
    for s in range(4):
        poolW[p, s, s, (p % 64) // 2] = 0.25
    poolW = poolW.reshape(128, 512)
    # classifier lhsT: wqT[p, c, cls] = w_pred[cls, d*1024 + (4g+s)*32 + w2]
    # with c = d*8+g, p = s*32+w2
    wq = w_pred.reshape(NCLS, D, 8, 4, 32)          # [cls, d, g, s, w2]
    wqT = np.transpose(wq, (3, 4, 1, 2, 0)).reshape(128, D * 8, NCLS)
    wqT = np.ascontiguousarray(wqT.reshape(128, D * 8 * NCLS)).astype(np.float32)
    bpred = b_pred.reshape(NCLS, 1).astype(np.float32)
    return pconst, pw, ones4k, ident, poolW, wqT, bpred


def _build_kernel(c1, c2, dy1, dx1, dy2, dx2):
    """Build + compile the per-core kernel. Fern geometry is baked into the
    selection matrix at trace time."""
    nc = bacc.Bacc("TRN2", num_devices=NCORES, num_swdge_queues=4)

    xp_p = nc.declare_dram_parameter("xp", [NI, C, HP, HP], FP16, isOutput=False)
    pt_p = nc.declare_dram_parameter("pt", [M, PTROWS, 4 * D], F32, isOutput=False)
    pc_p = nc.declare_dram_parameter("pconst", [128, 32], F32, isOutput=False)
    pw_p = nc.declare_dram_parameter("pwsel", [128, M * K], FP16, isOutput=False)
    on_p = nc.declare_dram_parameter("ones4k", [1, NPX], FP16, isOutput=False)
    id_p = nc.declare_dram_parameter("ident", [128, 128], F32, isOutput=False)
    pl_p = nc.declare_dram_parameter("poolw", [128, 512], F32, isOutput=False)
    wq_p = nc.declare_dram_parameter("wqt", [128, D * 8 * NCLS], F32, isOutput=False)
    bp_p = nc.declare_dram_parameter("bpred", [NCLS, 1], F32, isOutput=False)
    out_p = nc.declare_dram_parameter("out", [NCLS, NI], F32, isOutput=True)

    with tile.TileContext(nc, num_cores=NCORES) as tc:
        with ExitStack() as ctx:
            cpool = ctx.enter_context(tc.tile_pool(name="consts", bufs=1))
            awp = ctx.enter_context(tc.tile_pool(name="allwin", bufs=2))
            bpxp = ctx.enter_context(tc.tile_pool(name="bpx", bufs=2))
            kp = ctx.enter_context(tc.tile_pool(name="kstage", bufs=1))
            mp = ctx.enter_context(tc.tile_pool(name="mstage", bufs=1))
            msc = ctx.enter_context(tc.tile_pool(name="mscratch", bufs=1))
            wp = ctx.enter_context(tc.tile_pool(name="wgidx", bufs=2))
            idxp = ctx.enter_context(tc.tile_pool(name="idx", bufs=2))
            vp = ctx.enter_context(tc.tile_pool(name="votes", bufs=8))
            wvp = ctx.enter_context(tc.tile_pool(name="wv", bufs=1))
            fp = ctx.enter_context(tc.tile_pool(name="feat", bufs=1))
            flp = ctx.enter_context(tc.tile_pool(name="flat", bufs=1))
            tps = ctx.enter_context(tc.tile_pool(name="tpsum", bufs=3, space="PSUM"))
            dps = ctx.enter_context(tc.tile_pool(name="dpsum", bufs=2, space="PSUM"))
            pps = ctx.enter_context(tc.tile_pool(name="ppsum", bufs=2, space="PSUM"))
            lps = ctx.enter_context(tc.tile_pool(name="lpsum", bufs=1, space="PSUM"))

            # ---- constants ----
            pconst = cpool.tile([128, 32], F32)
            nc.sync.dma_start(pconst[:], pc_p.ap())
            pwsel = cpool.tile([128, M * K], FP16)
            nc.sync.dma_start(pwsel[:], pw_p.ap())
            ones4k = cpool.tile([1, NPX], F32)
            nc.sync.dma_start(ones4k[:], on_p.ap())
            ident = cpool.tile([128, 128], F32)
            nc.sync.dma_start(ident[:], id_p.ap())
            poolw = cpool.tile([128, 512], F32)
            nc.sync.dma_start(poolw[:], pl_p.ap())
            wqt = cpool.tile([128, D * 8 * NCLS], F32)
            nc.sync.dma_start(wqt[:], wq_p.ap())
            bpred = cpool.tile([NCLS, 1], F32)
            nc.sync.dma_start(bpred[:], bp_p.ap())

            def bc10(col):  # [128,10] const col -> [128,NT*M,10] broadcast
                v = pconst[:, col:col + 10]
                return v.unsqueeze(1).broadcast_to([128, NT * M, K])

            pow2b, iotab, iota16b = bc10(0), bc10(10), bc10(20)

            feat = fp.tile([128, NI, NT, D], F32)
            flatbuf = flp.tile([128, D, 8, NI], F32)

            bpx_tiles = [None] * NI

            def emit_stage_a(img):
                # ---- stage A: all-windows load + select/transpose matmul ----
                allwin = awp.tile([128, NPX], FP16, tag="allwin")
                for c in range(C):
                    for dy in range(L):
                        src = bass.AP(xp_p.ap().tensor,
                                      (img * C + c) * HP * HP + dy * HP,
                                      [[1, L], [HP, H], [1, W]])
                        w0 = c * 36 + dy * 6
                        nc.sync.dma_start(
                            allwin[w0:w0 + 6, :]
                            .rearrange("p (a b) -> p a b", a=H, b=W), src)
                nc.sync.dma_start(allwin[NWIN:NWIN + 1, :], on_p.ap())

                bpx = bpxp.tile([128, NT, M, K], F32, tag="bpx")
                bpx_tiles[img] = bpx
                done = 0
                while done < NT:
                    grp = min(4, NT - done)
                    tp = tps.tile([128, 4 * M * K], F32, tag="tp")
                    for i in range(grp):
                        t_ = done + i
                        nc.tensor.matmul(
                            tp[:, i * 80:(i + 1) * 80],
                            allwin[0:NWIN + 1, t_ * 128:(t_ + 1) * 128],
                            pwsel[0:NWIN + 1, :], start=True, stop=True)
                    nc.scalar.activation(
                        bpx[:, done:done + grp, :, :]
                        .rearrange("p t m k -> p (t m k)"),
                        tp[:, 0:80 * grp], ACT.Sigmoid)
                    done += grp

            emit_stage_a(0)
            for img in range(NI):
                bpx = bpx_tiles[img]
                # ---- stage B: per-site bit stats (pixel layout) ----
                TM = NT * M
                dt_ = kp.tile([128, TM, K], F32, tag="dt")
                eq = kp.tile([128, TM, K], F32, tag="eq")
                s1 = kp.tile([128, TM, K], F32, tag="s1")
                dm = dt_

                base = mp.tile([128, TM], F32, tag="base")
                k1t = mp.tile([128, TM], F32, tag="k1t")
                p2a = mp.tile([128, TM], F32, tag="p2a")
                bat = mp.tile([128, TM], F32, tag="bat")
                k2t = mp.tile([128, TM], F32, tag="k2t")
                p2b = mp.tile([128, TM], F32, tag="p2b")
                bbt = mp.tile([128, TM], F32, tag="bbt")
                mred = mp.tile([128, TM], F32, tag="mred")

                bpx3 = bpx[:].rearrange("p t m k -> p (t m) k")
                bpxF = bpx[:].rearrange("p t m k -> p (t m k)")

                def fl(t):  # [128,TM,K] -> flat 2D
                    return t[:].rearrange("p s k -> p (s k)")

                def bcm(t):  # [128,TM] -> broadcast over K
                    return t[:].unsqueeze(-1).broadcast_to([128, TM, K])

                nc.scalar.activation(fl(dt_), bpxF, ACT.Abs, bias=pconst[:, 30:31], scale=1.0)
                nc.scalar.activation(fl(s1), bpxF, ACT.Sign, bias=pconst[:, 30:31], scale=1.0)
                nc.scalar.activation(fl(s1), fl(s1), ACT.Relu)       # h
                nc.vector.tensor_mul(eq[:], s1[:], pow2b)
                nc.vector.tensor_reduce(base[:], eq[:], mybir.AxisListType.X, ALU.add)
                # first ambiguous bit
                nc.vector.tensor_reduce(mred[:], dt_[:], mybir.AxisListType.X, ALU.min)
                nc.vector.tensor_tensor(eq[:], dt_[:], bcm(mred), ALU.is_equal)
                nc.vector.scalar_tensor_tensor(s1[:], eq[:], -16.0, iota16b,
                                               ALU.mult, ALU.add)
                nc.vector.tensor_reduce(k1t[:], s1[:], mybir.AxisListType.X, ALU.min)
                nc.vector.tensor_tensor(eq[:], iotab, bcm(k1t), ALU.is_equal)
                nc.vector.tensor_mul(s1[:], eq[:], pow2b)
                nc.vector.tensor_reduce(p2a[:], s1[:], mybir.AxisListType.X, ALU.add)
                nc.vector.tensor_mul(s1[:], eq[:], bpx3)
                nc.vector.tensor_reduce(bat[:], s1[:], mybir.AxisListType.X, ALU.add)
                # second ambiguous bit
                nc.vector.scalar_tensor_tensor(dm[:], eq[:], 8.0, dt_[:],
                                               ALU.mult, ALU.add)
                nc.vector.tensor_reduce(mred[:], dm[:], mybir.AxisListType.X, ALU.min)
                nc.vector.tensor_tensor(eq[:], dm[:], bcm(mred), ALU.is_equal)
                nc.vector.scalar_tensor_tensor(s1[:], eq[:], -16.0, iota16b,
                                               ALU.mult, ALU.add)
                nc.vector.tensor_reduce(k2t[:], s1[:], mybir.AxisListType.X, ALU.min)
                nc.vector.tensor_tensor(eq[:], iotab, bcm(k2t), ALU.is_equal)
                nc.vector.tensor_mul(s1[:], eq[:], pow2b)
                nc.vector.tensor_reduce(p2b[:], s1[:], mybir.AxisListType.X, ALU.add)
                nc.vector.tensor_mul(s1[:], eq[:], bpx3)
                nc.vector.tensor_reduce(bbt[:], s1[:], mybir.AxisListType.X, ALU.add)

                # ---- stage C: pair/word/weight math ([128, NT*M]) ----
                def mt(tag):
                    return msc.tile([128, NT * M], F32, tag=tag, name=tag)

                klo, khi = mt("klo"), mt("khi")
                p2lo, p2hi = mt("p2lo"), mt("p2hi")
                blo, bhi = mt("blo"), mt("bhi")
                sc1, sc2 = mt("sc1"), mt("sc2")
                nc.vector.tensor_tensor(klo[:], k1t[:], k2t[:], ALU.min)
                nc.vector.tensor_tensor(khi[:], k1t[:], k2t[:], ALU.max)
                nc.vector.tensor_tensor(p2lo[:], p2a[:], p2b[:], ALU.min)
                nc.vector.tensor_tensor(p2hi[:], p2a[:], p2b[:], ALU.max)
                nc.vector.tensor_tensor(sc1[:], k1t[:], k2t[:], ALU.is_gt)   # swap
                nc.vector.tensor_tensor(sc2[:], bbt[:], bat[:], ALU.subtract)
                nc.vector.tensor_mul(sc1[:], sc1[:], sc2[:])
                nc.vector.tensor_tensor(blo[:], bat[:], sc1[:], ALU.add)
                nc.vector.tensor_tensor(sc2[:], bat[:], bbt[:], ALU.add)
                nc.vector.tensor_tensor(bhi[:], sc2[:], blo[:], ALU.subtract)
                # pid = khi*(khi-1)/2 + klo ; gidx = pid*256 + base8
                nc.vector.scalar_tensor_tensor(sc1[:], khi[:], -1.0, khi[:],
                                               ALU.add, ALU.mult)
                nc.vector.scalar_tensor_tensor(sc1[:], sc1[:], 0.5, klo[:],
                                               ALU.mult, ALU.add)            # pid
                # base_clear
                nc.vector.tensor_scalar(sc2[:], blo[:], 0.5, None, ALU.is_gt)
                nc.vector.tensor_mul(sc2[:], sc2[:], p2lo[:])
                nc.vector.tensor_tensor(base[:], base[:], sc2[:], ALU.subtract)
                nc.vector.tensor_scalar(sc2[:], bhi[:], 0.5, None, ALU.is_gt)
                nc.vector.tensor_mul(sc2[:], sc2[:], p2hi[:])
                nc.vector.tensor_tensor(base[:], base[:], sc2[:], ALU.subtract)
                # base8 = pack(base; p2lo, p2hi) -- int domain, mod 2^k = AND(2^k-1)

                def mti(tag):
                    return msc.tile([128, NT * M], I32, tag=tag, name=tag)

                bci, loi, hii = mti("bci"), mti("loi"), mti("hii")
                t1i, t2i = mti("t1i"), mti("t2i")
                lowv = mt("lowv")
                nc.vector.tensor_copy(bci[:], base[:])
                nc.vector.tensor_copy(loi[:], p2lo[:])
                nc.vector.tensor_copy(hii[:], p2hi[:])
                nc.vector.tensor_scalar(loi[:], loi[:], -1, None, ALU.add)
                nc.vector.tensor_scalar(hii[:], hii[:], -1, None, ALU.add)
                nc.vector.tensor_tensor(loi[:], bci[:], loi[:], ALU.bitwise_and)
                nc.vector.tensor_tensor(hii[:], bci[:], hii[:], ALU.bitwise_and)
                nc.vector.tensor_tensor(t1i[:], hii[:], loi[:], ALU.subtract)
                nc.vector.tensor_scalar(t1i[:], t1i[:], 1, None, ALU.arith_shift_right)
                nc.vector.tensor_tensor(t2i[:], bci[:], hii[:], ALU.subtract)
                nc.vector.tensor_scalar(t2i[:], t2i[:], 2, None, ALU.arith_shift_right)
                nc.vector.tensor_tensor(t1i[:], t1i[:], t2i[:], ALU.add)
                nc.vector.tensor_tensor(t1i[:], t1i[:], loi[:], ALU.add)     # base8 int
                nc.vector.tensor_copy(lowv[:], t1i[:])                       # base8 f32
                nc.vector.scalar_tensor_tensor(lowv[:], sc1[:], 256.0, lowv[:],
                                               ALU.mult, ALU.add)            # gidx
                # weights, m-major: [128, M, NT, 4]
                wt = wp.tile([128, M, NT, 4], F32, tag="wt")
                nc.vector.tensor_scalar(sc1[:], blo[:], -1.0, 1.0, ALU.mult, ALU.add)
                nc.vector.tensor_scalar(sc2[:], bhi[:], -1.0, 1.0, ALU.mult, ALU.add)

                def wslot(jj):  # [128, M, NT] view ordered as (t, m)
                    return wt[:, :, :, jj].rearrange("p m t -> p t m")

                def v3(t):  # [128,TM] -> [128, NT, M]
                    return t[:].rearrange("p (t m) -> p t m", t=NT, m=M)

                nc.vector.tensor_mul(wslot(0), v3(sc1), v3(sc2))
                nc.vector.tensor_mul(wslot(1), v3(blo), v3(sc2))
                nc.vector.tensor_mul(wslot(2), v3(sc1), v3(bhi))
                nc.vector.tensor_mul(wslot(3), v3(blo), v3(bhi))

                # ---- stage D: PE fold of idxs to wrapped [16, n/16] layout ----
                idxt = idxp.tile([128, M, NT, 8], I16, tag="idxt")
                for f in range(8):
                    fold = dps.tile([16, NT * M], F32, tag="fold")
                    nc.tensor.matmul(fold[:], ident[:, 16 * f:16 * (f + 1)],
                                     lowv[:], start=True, stop=True)
                    nc.scalar.copy(
                        idxt[0:16, :, :, f],
                        fold[:].rearrange("q (t m) -> q m t", t=NT, m=M))
                for g in range(1, 8):
                    nc.gpsimd.dma_start(idxt[16 * g:16 * (g + 1), :, :, :],
                                        idxt[0:16, :, :, :])

                # ---- stage E: gather + weighted vote reduce ----
                for m in range(M):
                    v = vp.tile([128, NT, 4, D], F32, tag="v")
                    if os.environ.get("KBISECT") == "nogather":
                        nc.vector.memset(v[:].rearrange("p t j d -> p (t j d)"), 0)
                    else:
                        nc.gpsimd.dma_gather(
                            out_ap=v[:].rearrange("p t j d -> p t (j d)"),
                            in_ap=pt_p.ap()[m],
                            idxs_ap=idxt[:, m, :, :],
                            num_idxs=NPX,
                            num_idxs_reg=NPX,
                            elem_size=4 * D,
                            single_packet=False,
                            queue_num=m % 4,
                        )
                    wv = wvp.tile([128, NT, 4, D], F32, tag="wv")
                    wb = wt[:, m, :, :].rearrange("p t j -> p (t j)") \
                        .unsqueeze(-1).broadcast_to([128, NT * 4, D])
                    nc.vector.tensor_tensor(
                        wv[:].rearrange("p t j d -> p (t j) d"),
                        v[:].rearrange("p t j d -> p (t j) d"), wb, ALU.mult)
                    nc.vector.tensor_add(
                        wv[:, :, 0:2, :].rearrange("p t j d -> p t (j d)"),
                        wv[:, :, 0:2, :].rearrange("p t j d -> p t (j d)"),
                        wv[:, :, 2:4, :].rearrange("p t j d -> p t (j d)"))
                    nc.vector.tensor_add(wv[:, :, 0, :], wv[:, :, 0, :],
                                         wv[:, :, 1, :])
                    if m == 0:
                        nc.scalar.copy(feat[:, img], wv[:, :, 0, :])
                    else:
                        nc.vector.tensor_add(feat[:, img], feat[:, img],
                                             wv[:, :, 0, :])

                if img + 1 < NI:
                    emit_stage_a(img + 1)

                # ---- stage F: 2x2 avg pool via PE ----
                pps_t = pps.tile([128, 8, D], F32, tag="pp")
                for g in range(8):
                    for s in range(4):
                        nc.tensor.matmul(
                            pps_t[:, g, :],
                            poolw[:, s * 128:(s + 1) * 128],
                            feat[:, img, 4 * g + s, :],
                            start=(s == 0), stop=(s == 3))
                nc.scalar.copy(
                    flatbuf[:, :, :, img],
                    pps_t[:].rearrange("p g d -> p d g"))

            # ---- classifier ----
            lg = lps.tile([NCLS, NI], F32)
            wqv = wqt[:].rearrange("p (c l) -> p c l", c=D * 8, l=NCLS)
            flv = flatbuf[:].rearrange("p d g i -> p (d g) i")
            for c_ in range(D * 8):
                nc.tensor.matmul(lg[:], wqv[:, c_, :], flv[:, c_, :],
                                 start=(c_ == 0), stop=(c_ == D * 8 - 1))
            lsb = flp.tile([NCLS, NI], F32)
            nc.scalar.activation(lsb[:], lg[:], ACT.Identity, bias=bpred[:], scale=1.0)
            nc.sync.dma_start(out_p.ap(), lsb[:])

    nc.compile()
    return nc


_CACHE: dict = {}


def _get_kernel(c1, c2, dy1, dx1, dy2, dx2):
    key = (c1.tobytes(), c2.tobytes(), dy1.tobytes(), dx1.tobytes(),
           dy2.tobytes(), dx2.tobytes())
    if key not in _CACHE:
        _CACHE[key] = _build_kernel(c1, c2, dy1, dx1, dy2, dx2)
    return _CACHE[key]


def kernel(x, c1, c2, dy1, dx1, dy2, dx2, thresholds, table, w_pred, b_pred):
    x = np.asarray(x, dtype=np.float32)
    c1, c2 = np.asarray(c1, np.int32), np.asarray(c2, np.int32)
    dy1, dx1 = np.asarray(dy1, np.int32), np.asarray(dx1, np.int32)
    dy2, dx2 = np.asarray(dy2, np.int32), np.asarray(dx2, np.int32)
    thresholds = np.asarray(thresholds, np.float32)
    table = np.asarray(table, np.float32)
    w_pred = np.asarray(w_pred, np.float32)
    b_pred = np.asarray(b_pred, np.float32)

    nc = _get_kernel(c1, c2, dy1, dx1, dy2, dx2)

    xp = np.pad(x, ((0, 0), (0, 0), (0, L - 1), (0, L - 1)))
    PT = _build_pair_table(table)
    pconst, pwsel, ones4k, ident, poolW, wqT, bpred = _host_consts(
        thresholds, w_pred, b_pred, c1, c2, dy1, dx1, dy2, dx2)

    in_maps = []
    for c in range(NCORES):
        in_maps.append(dict(
            xp=np.ascontiguousarray(xp[c * NI:(c + 1) * NI]),
            pt=PT, pconst=pconst, pwsel=pwsel, ones4k=ones4k, ident=ident,
            poolw=poolW, wqt=wqT, bpred=bpred,
        ))
    res = run_bass_kernel_spmd(nc, in_maps, core_ids=list(range(NCORES)))
    outs = [r["out"].T for r in res.results]      # each [NI, NCLS]
    return np.concatenate(outs, axis=0).astype(np.float32)
